# revision 1
# baseline (speedup 1.0000x reference)
"""PointGNNConv (sum aggregation) on 8 Trainium2 NeuronCores.

Algebraic decomposition: with f_w = [f_w3; f_wx] (3+128 rows),
    msg_e = relu(edge_feat @ f_w + f_b)
          = relu(u[src_e] + v[dst_e])
    u_j = pos_j @ f_w3 + x_j @ f_wx
    v_i = (delta_i - pos_i) @ f_w3 + f_b
so the per-edge work reduces to two row gathers + add + relu, followed by a
segment-sum (edges sorted by dst).

Sharding: dst-range sharding — core c owns dst in [c*NPC, (c+1)*NPC).
Two NEFFs: phase A computes per-node u/v on each core's node slice (host
reassembles the full u table between NEFFs — pure row reordering, no FP);
phase B gathers u[src]/v[dst] per edge with the SWDGE dma_gather, applies
add+relu, segment-sums via per-chunk matmuls against on-chip-built selection
matrices, applies the g-MLP and residual.

Segment-sum: edges sorted by local dst, grouped into sections of 128 dsts;
per section a fixed number of 128-edge chunks (lo: src < 32768, hi: rest —
the SWDGE gather index is int16 so the u table is addressed as two halves).
Chunk matmul: out[feat, w] += msg_chunk[slot, feat]^T @ S_chunk[slot, w]
with S[slot, w] = (dst_local_in_section[slot] == w), accumulated in PSUM
over a supergroup of 4 sections (512 dsts), then evacuated to an aggT tile.
"""
import sys

sys.path.insert(0, "/opt/trn_rl_repo")

import numpy as np
import ml_dtypes

import concourse.bass as bass
import concourse.mybir as mybir
import concourse.tile as tile
from concourse import bacc
from concourse.bass_utils import run_bass_kernel_spmd

BF16 = mybir.dt.bfloat16
F32 = mybir.dt.float32
I16 = mybir.dt.int16
AF = mybir.ActivationFunctionType
ALU = mybir.AluOpType

NCORES = 8
PAD_PDL = 200.0  # pdl value for pad slots; never equals a real column id


class Cfg:
    def __init__(self, n, e, din, lo_rows, dt=512):
        self.N = n
        self.E = e
        self.DIN = din
        self.NPC = n // NCORES          # nodes (dsts) per core
        self.LO = lo_rows               # u-table split point (int16 idx limit)
        self.SEC = 128                  # dsts per section
        self.NSEC = -(-self.NPC // self.SEC)
        self.SG_SECS = 4                # sections per supergroup (psum window)
        self.NSG = -(-self.NSEC // self.SG_SECS)
        self.DT = dt                    # free-dim tile for phase A / C


def _dtiles(total, dt):
    return [(i, min(dt, total - i)) for i in range(0, total, dt)]


def _secs_of_sg(cfg, sg):
    s0 = sg * cfg.SG_SECS
    return s0, min(s0 + cfg.SG_SECS, cfg.NSEC)


# ---------------------------------------------------------------- phase A
def build_phase_a(cfg):
    nc = bacc.Bacc(num_devices=NCORES)
    D = cfg.DIN
    xT = nc.dram_tensor("xT", [D, cfg.NPC], BF16, kind="ExternalInput")
    posT = nc.dram_tensor("posT", [3, cfg.NPC], BF16, kind="ExternalInput")
    h_w1 = nc.dram_tensor("h_w1", [D, D], BF16, kind="ExternalInput")
    h_b1 = nc.dram_tensor("h_b1", [D, 1], F32, kind="ExternalInput")
    h_w2 = nc.dram_tensor("h_w2", [D, 3], BF16, kind="ExternalInput")
    h_b2 = nc.dram_tensor("h_b2", [3, 1], F32, kind="ExternalInput")
    f_w3 = nc.dram_tensor("f_w3", [3, D], BF16, kind="ExternalInput")
    f_wx = nc.dram_tensor("f_wx", [D, D], BF16, kind="ExternalInput")
    f_b = nc.dram_tensor("f_b", [D, 1], F32, kind="ExternalInput")
    uT = nc.dram_tensor("uT", [D, cfg.NPC], BF16, kind="ExternalOutput")
    vT = nc.dram_tensor("vT", [D, cfg.NPC], BF16, kind="ExternalOutput")

    with tile.TileContext(nc) as tc:
        with (
            tc.tile_pool(name="consts", bufs=1) as cp,
            tc.tile_pool(name="work", bufs=2) as wp,
            tc.tile_pool(name="psum", bufs=2, space="PSUM") as pp,
        ):
            xT_sb = cp.tile([D, cfg.NPC], BF16)
            nc.sync.dma_start(out=xT_sb[:], in_=xT[:])
            posT_sb = cp.tile([3, cfg.NPC], BF16)
            nc.sync.dma_start(out=posT_sb[:], in_=posT[:])
            w1_sb = cp.tile([D, D], BF16)
            nc.sync.dma_start(out=w1_sb[:], in_=h_w1[:])
            w2_sb = cp.tile([D, 3], BF16)
            nc.sync.dma_start(out=w2_sb[:], in_=h_w2[:])
            fw3_sb = cp.tile([3, D], BF16)
            nc.sync.dma_start(out=fw3_sb[:], in_=f_w3[:])
            fwx_sb = cp.tile([D, D], BF16)
            nc.sync.dma_start(out=fwx_sb[:], in_=f_wx[:])
            b1_sb = cp.tile([D, 1], F32)
            nc.sync.dma_start(out=b1_sb[:], in_=h_b1[:])
            b2_sb = cp.tile([3, 1], F32)
            nc.sync.dma_start(out=b2_sb[:], in_=h_b2[:])
            fb_sb = cp.tile([D, 1], F32)
            nc.sync.dma_start(out=fb_sb[:], in_=f_b[:])

            for off, w in _dtiles(cfg.NPC, cfg.DT):
                sl = slice(off, off + w)
                xt = xT_sb[:, sl]
                pt = posT_sb[:, sl]
                # t1 = relu(x @ h_w1 + h_b1)   [D, w]
                ps1 = pp.tile([D, cfg.DT], F32, tag="ps1")
                nc.tensor.matmul(out=ps1[:, :w], lhsT=w1_sb[:], rhs=xt,
                                 start=True, stop=True)
                t1 = wp.tile([D, cfg.DT], BF16, tag="t1")
                nc.scalar.activation(out=t1[:, :w], in_=ps1[:, :w], func=AF.Relu,
                                     bias=b1_sb[:])
                # delta = tanh(t1 @ h_w2 + h_b2)  [3, w]
                ps2 = pp.tile([3, cfg.DT], F32, tag="ps2")
                nc.tensor.matmul(out=ps2[:, :w], lhsT=w2_sb[:], rhs=t1[:, :w],
                                 start=True, stop=True)
                dmp = wp.tile([3, cfg.DT], BF16, tag="dmp")
                nc.scalar.activation(out=dmp[:, :w], in_=ps2[:, :w], func=AF.Tanh,
                                     bias=b2_sb[:])
                # diff = delta - pos  [3, w]
                nc.vector.tensor_tensor(out=dmp[:, :w], in0=dmp[:, :w],
                                        in1=pt, op=ALU.subtract)
                # u = pos @ f_w3 + x @ f_wx   [D, w]
                psu = pp.tile([D, cfg.DT], F32, tag="psu")
                nc.tensor.matmul(out=psu[:, :w], lhsT=fw3_sb[:], rhs=pt,
                                 start=True, stop=False)
                nc.tensor.matmul(out=psu[:, :w], lhsT=fwx_sb[:], rhs=xt,
                                 start=False, stop=True)
                ut = wp.tile([D, cfg.DT], BF16, tag="ut")
                nc.scalar.activation(out=ut[:, :w], in_=psu[:, :w], func=AF.Copy)
                nc.sync.dma_start(out=uT[:, sl], in_=ut[:, :w])
                # v = (delta - pos) @ f_w3 + f_b  [D, w]
                psv = pp.tile([D, cfg.DT], F32, tag="psv")
                nc.tensor.matmul(out=psv[:, :w], lhsT=fw3_sb[:], rhs=dmp[:, :w],
                                 start=True, stop=True)
                vt = wp.tile([D, cfg.DT], BF16, tag="vt")
                nc.scalar.activation(out=vt[:, :w], in_=psv[:, :w],
                                     func=AF.Identity, bias=fb_sb[:])
                nc.sync.dma_start(out=vT[:, sl], in_=vt[:, :w])
    nc.finalize()
    return nc


# ---------------------------------------------------------------- phase B
def build_phase_b(cfg, c_lo, c_hi):
    nc = bacc.Bacc(num_devices=NCORES)
    D = cfg.DIN
    CLH = c_lo + c_hi
    lo_cols = cfg.NSEC * c_lo * 8   # idx cols (16 idx per col) for lo stream
    hi_cols = cfg.NSEC * c_hi * 8

    u_tbl = nc.dram_tensor("u_tbl", [cfg.N, D], BF16, kind="ExternalInput")
    v_tbl = nc.dram_tensor("v_tbl", [cfg.NPC, D], BF16, kind="ExternalInput")
    xT = nc.dram_tensor("xTf", [D, cfg.NPC], F32, kind="ExternalInput")
    uidx_lo = nc.dram_tensor("uidx_lo", [128, lo_cols], I16, kind="ExternalInput")
    vidx_lo = nc.dram_tensor("vidx_lo", [128, lo_cols], I16, kind="ExternalInput")
    if c_hi:
        uidx_hi = nc.dram_tensor("uidx_hi", [128, hi_cols], I16,
                                 kind="ExternalInput")
        vidx_hi = nc.dram_tensor("vidx_hi", [128, hi_cols], I16,
                                 kind="ExternalInput")
    pdl = nc.dram_tensor("pdl", [128, cfg.NSEC * CLH], BF16, kind="ExternalInput")
    gw1 = nc.dram_tensor("gw1", [D, D], BF16, kind="ExternalInput")
    gb1 = nc.dram_tensor("gb1", [D, 1], F32, kind="ExternalInput")
    gw2 = nc.dram_tensor("gw2", [D, D], BF16, kind="ExternalInput")
    gb2 = nc.dram_tensor("gb2", [D, 1], F32, kind="ExternalInput")
    outT = nc.dram_tensor("outT", [D, cfg.NPC], F32, kind="ExternalOutput")

    iota = nc.inline_tensor(
        np.broadcast_to(np.arange(128, dtype=ml_dtypes.bfloat16), (128, 128)).copy(),
        name="iota",
    )

    agg_cols = cfg.NSEC * cfg.SEC  # padded agg width (multiple of 128)

    with tile.TileContext(nc) as tc:
        with (
            tc.tile_pool(name="consts", bufs=1) as cp,
            tc.tile_pool(name="gat", bufs=2) as gp,
            tc.tile_pool(name="sbld", bufs=2) as sp,
            tc.tile_pool(name="cwork", bufs=2) as wp,
            tc.tile_pool(name="psagg", bufs=2, space="PSUM") as pa,
            tc.tile_pool(name="psmlp", bufs=2, space="PSUM") as pm,
        ):
            iota_sb = cp.tile([128, 128], BF16)
            nc.sync.dma_start(out=iota_sb[:], in_=iota[:])
            pdl_sb = cp.tile([128, cfg.NSEC * CLH], BF16)
            nc.sync.dma_start(out=pdl_sb[:], in_=pdl[:])
            uilo_sb = cp.tile([128, lo_cols], I16)
            nc.sync.dma_start(out=uilo_sb[:], in_=uidx_lo[:])
            vilo_sb = cp.tile([128, lo_cols], I16)
            nc.sync.dma_start(out=vilo_sb[:], in_=vidx_lo[:])
            if c_hi:
                uihi_sb = cp.tile([128, hi_cols], I16)
                nc.sync.dma_start(out=uihi_sb[:], in_=uidx_hi[:])
                vihi_sb = cp.tile([128, hi_cols], I16)
                nc.sync.dma_start(out=vihi_sb[:], in_=vidx_hi[:])
            gw1_sb = cp.tile([D, D], BF16)
            nc.sync.dma_start(out=gw1_sb[:], in_=gw1[:])
            gw2_sb = cp.tile([D, D], BF16)
            nc.sync.dma_start(out=gw2_sb[:], in_=gw2[:])
            gb1_sb = cp.tile([D, 1], F32)
            nc.sync.dma_start(out=gb1_sb[:], in_=gb1[:])
            gb2_sb = cp.tile([D, 1], F32)
            nc.sync.dma_start(out=gb2_sb[:], in_=gb2[:])
            aggT = cp.tile([D, agg_cols], BF16)

            for sg in range(cfg.NSG):
                s0, s1 = _secs_of_sg(cfg, sg)
                secs = s1 - s0
                nlo = secs * c_lo * 128
                nhi = secs * c_hi * 128

                ulo = gp.tile([128, cfg.SG_SECS * c_lo, D], BF16, tag="ulo")
                nc.gpsimd.dma_gather(
                    ulo[:, :secs * c_lo, :], u_tbl[:cfg.LO, :],
                    uilo_sb[:, s0 * c_lo * 8:s1 * c_lo * 8], nlo, nlo, D,
                    single_packet=False)
                vlo = gp.tile([128, cfg.SG_SECS * c_lo, D], BF16, tag="vlo")
                nc.gpsimd.dma_gather(
                    vlo[:, :secs * c_lo, :], v_tbl[:],
                    vilo_sb[:, s0 * c_lo * 8:s1 * c_lo * 8], nlo, nlo, D,
                    single_packet=False)
                if c_hi:
                    uhi = gp.tile([128, cfg.SG_SECS * c_hi, D], BF16, tag="uhi")
                    nc.gpsimd.dma_gather(
                        uhi[:, :secs * c_hi, :], u_tbl[cfg.LO:, :],
                        uihi_sb[:, s0 * c_hi * 8:s1 * c_hi * 8], nhi, nhi, D,
                        single_packet=False)
                    vhi = gp.tile([128, cfg.SG_SECS * c_hi, D], BF16, tag="vhi")
                    nc.gpsimd.dma_gather(
                        vhi[:, :secs * c_hi, :], v_tbl[:],
                        vihi_sb[:, s0 * c_hi * 8:s1 * c_hi * 8], nhi, nhi, D,
                        single_packet=False)

                # msg = relu(u + v), in place in the u tiles
                mlo = ulo[:, :secs * c_lo, :].rearrange("p c f -> p (c f)")
                nc.vector.tensor_tensor(
                    out=mlo, in0=mlo,
                    in1=vlo[:, :secs * c_lo, :].rearrange("p c f -> p (c f)"),
                    op=ALU.add)
                nc.vector.tensor_relu(mlo, mlo)
                if c_hi:
                    mhi = uhi[:, :secs * c_hi, :].rearrange("p c f -> p (c f)")
                    nc.vector.tensor_tensor(
                        out=mhi, in0=mhi,
                        in1=vhi[:, :secs * c_hi, :].rearrange("p c f -> p (c f)"),
                        op=ALU.add)
                    nc.vector.tensor_relu(mhi, mhi)

                # selection matrices for every chunk of this supergroup
                st = sp.tile([128, cfg.SG_SECS * CLH, 128], BF16, tag="st")
                for k in range(secs * CLH):
                    col = s0 * CLH + k
                    nc.vector.tensor_tensor(
                        out=st[:, k, :],
                        in0=pdl_sb[:, col:col + 1].to_broadcast([128, 128]),
                        in1=iota_sb[:],
                        op=ALU.is_equal)

                # segment-sum: psum[feat, w] += msg^T @ S per chunk
                ps = pa.tile([D, cfg.SG_SECS * cfg.SEC], F32, tag="psagg")
                for j in range(secs):
                    osl = slice(j * cfg.SEC, (j + 1) * cfg.SEC)
                    for t in range(c_lo):
                        nc.tensor.matmul(
                            out=ps[:, osl],
                            lhsT=ulo[:, j * c_lo + t, :],
                            rhs=st[:, j * CLH + t, :],
                            start=(t == 0), stop=(c_hi == 0 and t == c_lo - 1))
                    for t in range(c_hi):
                        nc.tensor.matmul(
                            out=ps[:, osl],
                            lhsT=uhi[:, j * c_hi + t, :],
                            rhs=st[:, j * CLH + c_lo + t, :],
                            start=False, stop=(t == c_hi - 1))
                nc.scalar.activation(
                    out=aggT[:, s0 * cfg.SEC:s1 * cfg.SEC],
                    in_=ps[:, :secs * cfg.SEC], func=AF.Copy)

            # phase C: out = x + relu(relu(agg @ g_w1 + g_b1) @ g_w2 + g_b2)
            for off, w in _dtiles(cfg.NPC, cfg.DT):
                sl = slice(off, off + w)
                ph1 = pm.tile([D, cfg.DT], F32, tag="ph1")
                nc.tensor.matmul(out=ph1[:, :w], lhsT=gw1_sb[:],
                                 rhs=aggT[:, sl], start=True, stop=True)
                h1 = wp.tile([D, cfg.DT], BF16, tag="h1")
                nc.scalar.activation(out=h1[:, :w], in_=ph1[:, :w], func=AF.Relu,
                                     bias=gb1_sb[:])
                ph2 = pm.tile([D, cfg.DT], F32, tag="ph2")
                nc.tensor.matmul(out=ph2[:, :w], lhsT=gw2_sb[:],
                                 rhs=h1[:, :w], start=True, stop=True)
                h2 = wp.tile([D, cfg.DT], F32, tag="h2")
                nc.scalar.activation(out=h2[:, :w], in_=ph2[:, :w], func=AF.Relu,
                                     bias=gb2_sb[:])
                xt = wp.tile([D, cfg.DT], F32, tag="xt")
                nc.sync.dma_start(out=xt[:, :w], in_=xT[:, sl])
                nc.vector.tensor_tensor(out=h2[:, :w], in0=h2[:, :w],
                                        in1=xt[:, :w], op=ALU.add)
                nc.sync.dma_start(out=outT[:, sl], in_=h2[:, :w])
    nc.finalize()
    return nc


# ------------------------------------------------------------ host side
def _wrap_idx(vals):
    """[n] int16 -> [128, n/16] wrapped (16 partitions) + replicated x8."""
    a = np.asarray(vals, dtype=np.int16).reshape(-1, 16).T  # [16, n/16]
    return np.ascontiguousarray(np.tile(a, (8, 1)))


def _preprocess(cfg, edge_index):
    """Sort/bucket edges per core; build idx + pdl arrays.

    Returns (c_lo, c_hi, per_core list of dicts).
    """
    src = np.asarray(edge_index[0], dtype=np.int64)
    dst = np.asarray(edge_index[1], dtype=np.int64)
    order = np.argsort(dst, kind="stable")
    src, dst = src[order], dst[order]
    core = dst // cfg.NPC
    bounds = np.searchsorted(core, np.arange(NCORES + 1))

    per_core = []
    for c in range(NCORES):
        lo_, hi_ = bounds[c], bounds[c + 1]
        s, d = src[lo_:hi_], dst[lo_:hi_] - c * cfg.NPC
        sec = d // cfg.SEC
        is_lo = s < cfg.LO
        per_core.append((s, d, sec, is_lo))

    def sec_counts(c, want_lo):
        s, d, sec, is_lo = per_core[c]
        m = is_lo if want_lo else ~is_lo
        return np.bincount(sec[m], minlength=cfg.NSEC)

    c_lo = max(1, max(int(np.max(np.ceil(sec_counts(c, True) / 128)))
                      for c in range(NCORES)))
    has_hi = cfg.LO < cfg.N
    c_hi = (max(1, max(int(np.max(np.ceil(sec_counts(c, False) / 128)))
                       for c in range(NCORES)))) if has_hi else 0

    data = []
    for c in range(NCORES):
        s, d, sec, is_lo = per_core[c]
        CLH = c_lo + c_hi
        pdl = np.full((128, cfg.NSEC * CLH), PAD_PDL, np.float32)

        def build(mask, cap, tbl_off):
            uidx = np.zeros((cfg.NSEC, cap * 128), np.int16)
            vidx = np.zeros((cfg.NSEC, cap * 128), np.int16)
            pcol = np.full((cfg.NSEC, cap, 128), PAD_PDL, np.float32)
            ss, dd, qq = s[mask], d[mask], sec[mask]
            for j in range(cfg.NSEC):
                m = qq == j
                n = int(m.sum())
                assert n <= cap * 128
                uidx[j, :n] = (ss[m] - tbl_off).astype(np.int16)
                vidx[j, :n] = dd[m].astype(np.int16)
                loc = (dd[m] - j * cfg.SEC).astype(np.float32)
                flat = pcol[j].reshape(-1)
                flat[:n] = loc
            return uidx, vidx, pcol

        ulo, vlo, plo = build(is_lo, c_lo, 0)
        if c_hi:
            uhi, vhi, phi = build(~is_lo, c_hi, cfg.LO)
        # pdl layout: per section: c_lo lo chunks then c_hi hi chunks;
        # chunk t of section j -> column j*CLH + t; rows = slots
        for j in range(cfg.NSEC):
            pdl[:, j * CLH:j * CLH + c_lo] = plo[j].T.reshape(128, c_lo)
            if c_hi:
                pdl[:, j * CLH + c_lo:(j + 1) * CLH] = phi[j].T.reshape(128, c_hi)

        entry = {
            "uidx_lo": _wrap_idx(ulo.reshape(-1)),
            "vidx_lo": _wrap_idx(vlo.reshape(-1)),
            "pdl": pdl.astype(ml_dtypes.bfloat16),
        }
        if c_hi:
            entry["uidx_hi"] = _wrap_idx(uhi.reshape(-1))
            entry["vidx_hi"] = _wrap_idx(vhi.reshape(-1))
        data.append(entry)
    return c_lo, c_hi, data


def run(cfg, inputs, trace=False):
    """Full pipeline. inputs: dict as from setup_inputs (numpy)."""
    bf = ml_dtypes.bfloat16
    x = np.asarray(inputs["x"], np.float32)
    pos = np.asarray(inputs["pos"], np.float32)
    c_lo, c_hi, edata = _preprocess(cfg, np.asarray(inputs["edge_index"]))

    f_w = np.asarray(inputs["f_w"], np.float32) if "f_w" in inputs else None

    h_w1 = np.asarray(inputs["h_w1"], np.float32)
    h_b1 = np.asarray(inputs["h_b1"], np.float32)
    h_w2 = np.asarray(inputs["h_w2"], np.float32)
    h_b2 = np.asarray(inputs["h_b2"], np.float32)
    f_w = np.asarray(inputs["f_w"], np.float32)
    f_b = np.asarray(inputs["f_b"], np.float32)
    g_w1 = np.asarray(inputs["g_w1"], np.float32)
    g_b1 = np.asarray(inputs["g_b1"], np.float32)
    g_w2 = np.asarray(inputs["g_w2"], np.float32)
    g_b2 = np.asarray(inputs["g_b2"], np.float32)

    nc_a = build_phase_a(cfg)
    in_a = []
    for c in range(NCORES):
        sl = slice(c * cfg.NPC, (c + 1) * cfg.NPC)
        in_a.append({
            "xT": np.ascontiguousarray(x[sl].T.astype(bf)),
            "posT": np.ascontiguousarray(pos[sl].T.astype(bf)),
            "h_w1": h_w1.astype(bf), "h_b1": h_b1[:, None],
            "h_w2": h_w2.astype(bf), "h_b2": h_b2[:, None],
            "f_w3": f_w[:3].astype(bf), "f_wx": f_w[3:].astype(bf),
            "f_b": f_b[:, None],
        })
    res_a = run_bass_kernel_spmd(nc_a, in_a, core_ids=list(range(NCORES)),
                                 trace=trace)
    u_tbl = np.concatenate(
        [np.ascontiguousarray(r["uT"].T) for r in res_a.results], axis=0)
    v_tbls = [np.ascontiguousarray(r["vT"].T) for r in res_a.results]

    nc_b = build_phase_b(cfg, c_lo, c_hi)
    in_b = []
    for c in range(NCORES):
        sl = slice(c * cfg.NPC, (c + 1) * cfg.NPC)
        m = {
            "u_tbl": u_tbl, "v_tbl": v_tbls[c],
            "xTf": np.ascontiguousarray(x[sl].T),
            "gw1": g_w1.astype(bf), "gb1": g_b1[:, None],
            "gw2": g_w2.astype(bf), "gb2": g_b2[:, None],
        }
        m.update(edata[c])
        in_b.append(m)
    res_b = run_bass_kernel_spmd(nc_b, in_b, core_ids=list(range(NCORES)),
                                 trace=trace)
    out = np.concatenate(
        [np.ascontiguousarray(r["outT"].T) for r in res_b.results], axis=0)
    return out, (res_a, res_b)


DEFAULT_CFG = Cfg(n=50000, e=500000, din=128, lo_rows=32768)


def kernel(**inputs):
    out, _ = run(DEFAULT_CFG, inputs)
    return out.astype(np.float32)



# revision 2
# speedup vs baseline: 5.3216x; 5.3216x over previous
"""PointGNNConv (sum aggregation) on 8 Trainium2 NeuronCores.

Algebraic decomposition: with f_w = [f_w3; f_wx] (3+128 rows),
    msg_e = relu(edge_feat @ f_w + f_b) = relu(u[src_e] + v[dst_e])
    u_j = pos_j @ f_w3 + x_j @ f_wx
    v_i = (delta_i - pos_i) @ f_w3 + f_b

Sharding: dst-range sharding -- core c owns dst in [c*NPC, (c+1)*NPC).

Two NEFFs. Phase A computes per-node u/v (fp8) on each core's node slice.
Between NEFFs the host expands the tables into per-edge streams (pure row
gather / reordering, no FP): u_exp[e] = u[src_e], v_exp[e] = v[dst_e] with
edges sorted by dst and padded per 128-dst section to a fixed chunk count C.
Phase B then needs NO on-device gather at all: it streams u_exp/v_exp
contiguously (HWDGE, large descriptors -- the SWDGE per-edge gather of the
previous design serialized ~1.2ms of descriptor generation on GpSimd),
computes msg = relu(u+v) on DVE, segment-sums via per-chunk matmuls against
on-chip-built selection matrices (S[slot, w] = (dst_local[slot] == w)),
accumulated in PSUM over supergroups of 4 sections (512 dsts), then applies
the g-MLP + residual per supergroup (fused tail).

fp8 (e4m3) is used for the two big per-edge streams; msg/S and all matmuls
stay bf16, psum f32, residual + output f32.
"""
import sys

sys.path.insert(0, "/opt/trn_rl_repo")

import numpy as np
import ml_dtypes

import concourse.bass as bass
import concourse.mybir as mybir
import concourse.tile as tile
from concourse import bacc
from concourse.bass_utils import run_bass_kernel_spmd

BF16 = mybir.dt.bfloat16
F32 = mybir.dt.float32
FP8 = mybir.dt.float8e4
AF = mybir.ActivationFunctionType
ALU = mybir.AluOpType

NCORES = 8
FP8NP = ml_dtypes.float8_e4m3


class Cfg:
    def __init__(self, n, e, din, dt=512):
        self.N = n
        self.E = e
        self.DIN = din
        self.NPC = n // NCORES          # nodes (dsts) per core
        self.SEC = 128                  # dsts per section
        self.NSEC = -(-self.NPC // self.SEC)
        self.SG_SECS = 4                # sections per supergroup (psum window)
        self.NSG = -(-self.NSEC // self.SG_SECS)
        self.DT = dt                    # free-dim tile for phase A
        self.C = None                   # chunks per section (set from data)


def _dtiles(total, dt):
    return [(i, min(dt, total - i)) for i in range(0, total, dt)]


# ---------------------------------------------------------------- phase A
def build_phase_a(cfg):
    nc = bacc.Bacc(num_devices=NCORES)
    D = cfg.DIN
    xT = nc.dram_tensor("xT", [D, cfg.NPC], BF16, kind="ExternalInput")
    posT = nc.dram_tensor("posT", [3, cfg.NPC], BF16, kind="ExternalInput")
    h_w1 = nc.dram_tensor("h_w1", [D, D], BF16, kind="ExternalInput")
    h_b1 = nc.dram_tensor("h_b1", [D, 1], F32, kind="ExternalInput")
    h_w2 = nc.dram_tensor("h_w2", [D, 3], BF16, kind="ExternalInput")
    h_b2 = nc.dram_tensor("h_b2", [3, 1], F32, kind="ExternalInput")
    f_w3 = nc.dram_tensor("f_w3", [3, D], BF16, kind="ExternalInput")
    f_wx = nc.dram_tensor("f_wx", [D, D], BF16, kind="ExternalInput")
    f_b = nc.dram_tensor("f_b", [D, 1], F32, kind="ExternalInput")
    uT = nc.dram_tensor("uT", [D, cfg.NPC], FP8, kind="ExternalOutput")
    vT = nc.dram_tensor("vT", [D, cfg.NPC], FP8, kind="ExternalOutput")

    with tile.TileContext(nc) as tc:
        with (
            tc.tile_pool(name="consts", bufs=1) as cp,
            tc.tile_pool(name="work", bufs=2) as wp,
            tc.tile_pool(name="psum", bufs=2, space="PSUM") as pp,
        ):
            xT_sb = cp.tile([D, cfg.NPC], BF16)
            nc.sync.dma_start(out=xT_sb[:], in_=xT[:])
            posT_sb = cp.tile([3, cfg.NPC], BF16)
            nc.sync.dma_start(out=posT_sb[:], in_=posT[:])
            w1_sb = cp.tile([D, D], BF16)
            nc.sync.dma_start(out=w1_sb[:], in_=h_w1[:])
            w2_sb = cp.tile([D, 3], BF16)
            nc.sync.dma_start(out=w2_sb[:], in_=h_w2[:])
            fw3_sb = cp.tile([3, D], BF16)
            nc.sync.dma_start(out=fw3_sb[:], in_=f_w3[:])
            fwx_sb = cp.tile([D, D], BF16)
            nc.sync.dma_start(out=fwx_sb[:], in_=f_wx[:])
            b1_sb = cp.tile([D, 1], F32)
            nc.sync.dma_start(out=b1_sb[:], in_=h_b1[:])
            b2_sb = cp.tile([3, 1], F32)
            nc.sync.dma_start(out=b2_sb[:], in_=h_b2[:])
            fb_sb = cp.tile([D, 1], F32)
            nc.sync.dma_start(out=fb_sb[:], in_=f_b[:])

            for off, w in _dtiles(cfg.NPC, cfg.DT):
                sl = slice(off, off + w)
                xt = xT_sb[:, sl]
                pt = posT_sb[:, sl]
                # t1 = relu(x @ h_w1 + h_b1)   [D, w]
                ps1 = pp.tile([D, cfg.DT], F32, tag="ps1")
                nc.tensor.matmul(out=ps1[:, :w], lhsT=w1_sb[:], rhs=xt,
                                 start=True, stop=True)
                t1 = wp.tile([D, cfg.DT], BF16, tag="t1")
                nc.scalar.activation(out=t1[:, :w], in_=ps1[:, :w], func=AF.Relu,
                                     bias=b1_sb[:])
                # delta = tanh(t1 @ h_w2 + h_b2)  [3, w]
                ps2 = pp.tile([3, cfg.DT], F32, tag="ps2")
                nc.tensor.matmul(out=ps2[:, :w], lhsT=w2_sb[:], rhs=t1[:, :w],
                                 start=True, stop=True)
                dmp = wp.tile([3, cfg.DT], BF16, tag="dmp")
                nc.scalar.activation(out=dmp[:, :w], in_=ps2[:, :w], func=AF.Tanh,
                                     bias=b2_sb[:])
                # diff = delta - pos  [3, w]
                nc.vector.tensor_tensor(out=dmp[:, :w], in0=dmp[:, :w],
                                        in1=pt, op=ALU.subtract)
                # u = pos @ f_w3 + x @ f_wx   [D, w]
                psu = pp.tile([D, cfg.DT], F32, tag="psu")
                nc.tensor.matmul(out=psu[:, :w], lhsT=fw3_sb[:], rhs=pt,
                                 start=True, stop=False)
                nc.tensor.matmul(out=psu[:, :w], lhsT=fwx_sb[:], rhs=xt,
                                 start=False, stop=True)
                ut = wp.tile([D, cfg.DT], FP8, tag="ut")
                nc.vector.tensor_copy(out=ut[:, :w], in_=psu[:, :w])
                nc.sync.dma_start(out=uT[:, sl], in_=ut[:, :w])
                # v = (delta - pos) @ f_w3 + f_b  [D, w]
                psv = pp.tile([D, cfg.DT], F32, tag="psv")
                nc.tensor.matmul(out=psv[:, :w], lhsT=fw3_sb[:], rhs=dmp[:, :w],
                                 start=True, stop=True)
                vt = wp.tile([D, cfg.DT], FP8, tag="vt")
                nc.scalar.activation(out=vt[:, :w], in_=psv[:, :w],
                                     func=AF.Identity, bias=fb_sb[:])
                nc.sync.dma_start(out=vT[:, sl], in_=vt[:, :w])
    nc.finalize()
    return nc


# ---------------------------------------------------------------- phase B
def build_phase_b(cfg):
    nc = bacc.Bacc(num_devices=NCORES)
    D = cfg.DIN
    C = cfg.C
    NCH = cfg.NSEC * C              # chunks per core

    u_exp = nc.dram_tensor("u_exp", [128, NCH, D], FP8, kind="ExternalInput")
    v_exp = nc.dram_tensor("v_exp", [128, NCH, D], FP8, kind="ExternalInput")
    pdl = nc.dram_tensor("pdl", [128, NCH], BF16, kind="ExternalInput")
    xTf = nc.dram_tensor("xTf", [D, cfg.NPC], F32, kind="ExternalInput")
    gw1 = nc.dram_tensor("gw1", [D, D], BF16, kind="ExternalInput")
    gb1 = nc.dram_tensor("gb1", [D, 1], F32, kind="ExternalInput")
    gw2 = nc.dram_tensor("gw2", [D, D], BF16, kind="ExternalInput")
    gb2 = nc.dram_tensor("gb2", [D, 1], F32, kind="ExternalInput")
    outT = nc.dram_tensor("outT", [D, cfg.NPC], F32, kind="ExternalOutput")

    iota = nc.inline_tensor(
        np.broadcast_to(np.arange(128, dtype=ml_dtypes.bfloat16), (128, 128)).copy(),
        name="iota",
    )
    SGC = cfg.SG_SECS * C           # chunks per supergroup (max)

    with tile.TileContext(nc) as tc:
        with (
            tc.tile_pool(name="consts", bufs=1) as cp,
            tc.tile_pool(name="stream", bufs=2) as gp,
            tc.tile_pool(name="cwork", bufs=2) as wp,
            tc.tile_pool(name="psagg", bufs=2, space="PSUM") as pa,
            tc.tile_pool(name="psmlp", bufs=2, space="PSUM") as pm,
        ):
            iota_sb = cp.tile([128, 128], BF16)
            nc.sync.dma_start(out=iota_sb[:], in_=iota[:])
            pdl_sb = cp.tile([128, NCH], BF16)
            nc.sync.dma_start(out=pdl_sb[:], in_=pdl[:])
            gw1_sb = cp.tile([D, D], BF16)
            nc.sync.dma_start(out=gw1_sb[:], in_=gw1[:])
            gw2_sb = cp.tile([D, D], BF16)
            nc.sync.dma_start(out=gw2_sb[:], in_=gw2[:])
            gb1_sb = cp.tile([D, 1], F32)
            nc.sync.dma_start(out=gb1_sb[:], in_=gb1[:])
            gb2_sb = cp.tile([D, 1], F32)
            nc.sync.dma_start(out=gb2_sb[:], in_=gb2[:])

            for sg in range(cfg.NSG):
                s0 = sg * cfg.SG_SECS
                s1 = min(s0 + cfg.SG_SECS, cfg.NSEC)
                secs = s1 - s0
                nch = secs * C
                csl = slice(s0 * C, s1 * C)

                ue = gp.tile([128, SGC, D], FP8, tag="ue")
                nc.sync.dma_start(out=ue[:, :nch, :], in_=u_exp[:, csl, :])
                ve = gp.tile([128, SGC, D], FP8, tag="ve")
                nc.sync.dma_start(out=ve[:, :nch, :], in_=v_exp[:, csl, :])

                # msg = relu(u + v)  [slot, chunk, feat] -> bf16
                msg = wp.tile([128, SGC, D], BF16, tag="msg")
                mf = msg[:, :nch, :].rearrange("p c f -> p (c f)")
                nc.vector.tensor_tensor(
                    out=mf, in0=ue[:, :nch, :].rearrange("p c f -> p (c f)"),
                    in1=ve[:, :nch, :].rearrange("p c f -> p (c f)"),
                    op=ALU.add)
                nc.vector.tensor_relu(mf, mf)

                # selection matrices: S[slot, k, w] = (pdl[slot, k] == w)
                st = wp.tile([128, SGC, 128], BF16, tag="st")
                nc.vector.tensor_tensor(
                    out=st[:, :nch, :],
                    in0=pdl_sb[:, csl, None].to_broadcast([128, nch, 128]),
                    in1=iota_sb[:, None, :].to_broadcast([128, nch, 128]),
                    op=ALU.is_equal)

                # segment-sum: psum[feat, w] += msg_chunk^T @ S_chunk
                ps = pa.tile([D, cfg.SG_SECS * cfg.SEC], F32, tag="psagg")
                for j in range(secs):
                    osl = slice(j * cfg.SEC, (j + 1) * cfg.SEC)
                    for t in range(C):
                        nc.tensor.matmul(
                            out=ps[:, osl],
                            lhsT=msg[:, j * C + t, :],
                            rhs=st[:, j * C + t, :],
                            start=(t == 0), stop=(t == C - 1))
                aggt = wp.tile([D, cfg.SG_SECS * cfg.SEC], BF16, tag="aggt")
                nc.scalar.activation(out=aggt[:, :secs * cfg.SEC],
                                     in_=ps[:, :secs * cfg.SEC], func=AF.Copy)

                # fused tail: out = x + relu(relu(agg@g_w1+g_b1)@g_w2+g_b2)
                n0 = s0 * cfg.SEC
                w = min(cfg.NPC, s1 * cfg.SEC) - n0
                nsl = slice(n0, n0 + w)
                ph1 = pm.tile([D, cfg.SG_SECS * cfg.SEC], F32, tag="ph1")
                nc.tensor.matmul(out=ph1[:, :w], lhsT=gw1_sb[:],
                                 rhs=aggt[:, :w], start=True, stop=True)
                h1 = wp.tile([D, cfg.SG_SECS * cfg.SEC], BF16, tag="h1")
                nc.scalar.activation(out=h1[:, :w], in_=ph1[:, :w], func=AF.Relu,
                                     bias=gb1_sb[:])
                ph2 = pm.tile([D, cfg.SG_SECS * cfg.SEC], F32, tag="ph2")
                nc.tensor.matmul(out=ph2[:, :w], lhsT=gw2_sb[:],
                                 rhs=h1[:, :w], start=True, stop=True)
                h2 = wp.tile([D, cfg.SG_SECS * cfg.SEC], F32, tag="h2")
                nc.scalar.activation(out=h2[:, :w], in_=ph2[:, :w], func=AF.Relu,
                                     bias=gb2_sb[:])
                xt = wp.tile([D, cfg.SG_SECS * cfg.SEC], F32, tag="xt")
                nc.sync.dma_start(out=xt[:, :w], in_=xTf[:, nsl])
                nc.vector.tensor_tensor(out=h2[:, :w], in0=h2[:, :w],
                                        in1=xt[:, :w], op=ALU.add)
                nc.sync.dma_start(out=outT[:, nsl], in_=h2[:, :w])
    nc.finalize()
    return nc


# ------------------------------------------------------------ host side
def _preprocess(cfg, edge_index):
    """Sort edges by dst per core, pad per 128-dst section to C chunks.

    Sets cfg.C. Returns per-core (src_pad [NCH*128] int64 global node ids,
    dst_pad [NCH*128] int64 core-local node ids, pdl_w [128, NCH] bf16
    with -1 in pad slots, valid [NCH*128] bool).
    """
    src = np.asarray(edge_index[0], dtype=np.int64)
    dst = np.asarray(edge_index[1], dtype=np.int64)
    order = np.argsort(dst, kind="stable")
    src, dst = src[order], dst[order]
    core = dst // cfg.NPC
    bounds = np.searchsorted(core, np.arange(NCORES + 1))

    percore = []
    cmax = 1
    for c in range(NCORES):
        lo, hi = bounds[c], bounds[c + 1]
        s, d = src[lo:hi], dst[lo:hi] - c * cfg.NPC
        sec = d // cfg.SEC
        cnt = np.bincount(sec, minlength=cfg.NSEC)
        cmax = max(cmax, int(np.ceil(cnt.max() / 128)))
        percore.append((s, d, sec, cnt))
    cfg.C = cmax
    NCH = cfg.NSEC * cfg.C

    out = []
    for c in range(NCORES):
        s, d, sec, cnt = percore[c]
        start = np.zeros(cfg.NSEC, np.int64)
        np.cumsum(cnt[:-1], out=start[1:])
        rank = np.arange(len(d)) - start[sec]          # rank within section
        slot = (sec * cfg.C + rank // 128) * 128 + rank % 128
        src_pad = np.zeros(NCH * 128, np.int64)
        dst_pad = np.zeros(NCH * 128, np.int64)
        pdl_flat = np.full(NCH * 128, -1.0, np.float32)
        src_pad[slot] = s
        dst_pad[slot] = d
        pdl_flat[slot] = (d % cfg.SEC).astype(np.float32)
        pdl_w = np.ascontiguousarray(
            pdl_flat.reshape(NCH, 128).T).astype(ml_dtypes.bfloat16)
        out.append((src_pad, dst_pad, pdl_w))
    return out


def _wrap_rows(rows, nch):
    """[NCH*128, D] -> [128, NCH, D] (slot p of chunk c = row c*128+p)."""
    return np.ascontiguousarray(rows.reshape(nch, 128, -1).transpose(1, 0, 2))


def run(cfg, inputs, trace=False):
    """Full pipeline. inputs: dict as from setup_inputs (numpy)."""
    bf = ml_dtypes.bfloat16
    x = np.asarray(inputs["x"], np.float32)
    pos = np.asarray(inputs["pos"], np.float32)
    edata = _preprocess(cfg, np.asarray(inputs["edge_index"]))
    NCH = cfg.NSEC * cfg.C

    h_w1 = np.asarray(inputs["h_w1"], np.float32)
    h_b1 = np.asarray(inputs["h_b1"], np.float32)
    h_w2 = np.asarray(inputs["h_w2"], np.float32)
    h_b2 = np.asarray(inputs["h_b2"], np.float32)
    f_w = np.asarray(inputs["f_w"], np.float32)
    f_b = np.asarray(inputs["f_b"], np.float32)
    g_w1 = np.asarray(inputs["g_w1"], np.float32)
    g_b1 = np.asarray(inputs["g_b1"], np.float32)
    g_w2 = np.asarray(inputs["g_w2"], np.float32)
    g_b2 = np.asarray(inputs["g_b2"], np.float32)

    nc_a = build_phase_a(cfg)
    in_a = []
    for c in range(NCORES):
        sl = slice(c * cfg.NPC, (c + 1) * cfg.NPC)
        in_a.append({
            "xT": np.ascontiguousarray(x[sl].T.astype(bf)),
            "posT": np.ascontiguousarray(pos[sl].T.astype(bf)),
            "h_w1": h_w1.astype(bf), "h_b1": h_b1[:, None],
            "h_w2": h_w2.astype(bf), "h_b2": h_b2[:, None],
            "f_w3": f_w[:3].astype(bf), "f_wx": f_w[3:].astype(bf),
            "f_b": f_b[:, None],
        })
    res_a = run_bass_kernel_spmd(nc_a, in_a, core_ids=list(range(NCORES)),
                                 trace=trace)
    # u table node-major over ALL nodes; v tables per-core node-major
    u_nm = np.concatenate(
        [np.ascontiguousarray(np.asarray(r["uT"]).T) for r in res_a.results],
        axis=0)
    v_nms = [np.ascontiguousarray(np.asarray(r["vT"]).T) for r in res_a.results]

    nc_b = build_phase_b(cfg)
    in_b = []
    for c in range(NCORES):
        sl = slice(c * cfg.NPC, (c + 1) * cfg.NPC)
        src_pad, dst_pad, pdl_w = edata[c]
        in_b.append({
            "u_exp": _wrap_rows(u_nm[src_pad], NCH),
            "v_exp": _wrap_rows(v_nms[c][dst_pad], NCH),
            "pdl": pdl_w,
            "xTf": np.ascontiguousarray(x[sl].T),
            "gw1": g_w1.astype(bf), "gb1": g_b1[:, None],
            "gw2": g_w2.astype(bf), "gb2": g_b2[:, None],
        })
    res_b = run_bass_kernel_spmd(nc_b, in_b, core_ids=list(range(NCORES)),
                                 trace=trace)
    out = np.concatenate(
        [np.ascontiguousarray(np.asarray(r["outT"]).T) for r in res_b.results],
        axis=0)
    return out, (res_a, res_b)


DEFAULT_CFG = Cfg(n=50000, e=500000, din=128)


def kernel(**inputs):
    out, _ = run(DEFAULT_CFG, inputs)
    return out.astype(np.float32)


# revision 6
# speedup vs baseline: 6.5045x; 1.2223x over previous
"""PointGNNConv (sum aggregation) on 8 Trainium2 NeuronCores.

Algebraic decomposition: with f_w = [f_w3; f_wx] (3+128 rows),
    msg_e = relu(edge_feat @ f_w + f_b) = relu(u[src_e] + v[dst_e])
    u_j = pos_j @ f_w3 + x_j @ f_wx
    v_i = (delta_i - pos_i) @ f_w3 + f_b

Sharding: dst-range sharding -- core c owns dst in [c*NPC, (c+1)*NPC).

Two NEFFs. Phase A computes per-node u/v (bf16) on each core's node slice.
Between NEFFs the host expands the tables into per-edge streams (pure row
gather / reordering, no FP) so phase B needs NO on-device gather (the SWDGE
per-edge gather of the original design serialized ~1.2ms of descriptor
generation on GpSimd).

Phase B edge layout (per core, edges sorted by dst, sections of 128 dsts):
 - DENSE: the first T edges of each dst go to column-aligned chunks -- slot
   p of dense chunk r holds the r-th edge of dst (sec_base+p). The add of
   v[dst] uses the *unexpanded* per-section v tile broadcast across chunks
   (no v stream), and the segment-sum over chunks is a DVE pairwise tree
   followed by ONE identity matmul per section (psum transpose-accumulate).
 - OVERFLOW: edges beyond T per dst (25% here) go to packed chunks with a
   dst-local label (pdl); selection matrices S[slot,w] = (pdl[slot]==w) are
   built on GpSimd (is_equal) and matmul-accumulated into the same psum
   window.
Everything is bf16 (DVE 2x/4x fast modes need 2-byte packed operands; fp8
runs at base rate), psum f32, output f32. The g-MLP + residual tail is
fused per supergroup (4 sections / 512 dsts).
"""
import sys

sys.path.insert(0, "/opt/trn_rl_repo")

import numpy as np
import ml_dtypes

import concourse.bass as bass
import concourse.mybir as mybir
import concourse.tile as tile
from concourse import bacc
from concourse.bass_utils import run_bass_kernel_spmd

BF16 = mybir.dt.bfloat16
F32 = mybir.dt.float32
AF = mybir.ActivationFunctionType
ALU = mybir.AluOpType

NCORES = 8
BF = ml_dtypes.bfloat16


class Cfg:
    def __init__(self, n, e, din, dt=512, t_dense=7):
        self.N = n
        self.E = e
        self.DIN = din
        self.NPC = n // NCORES          # nodes (dsts) per core
        self.SEC = 128                  # dsts per section
        self.NSEC = -(-self.NPC // self.SEC)
        self.SG_SECS = 4                # sections per supergroup (psum window)
        self.NSG = -(-self.NSEC // self.SG_SECS)
        self.DT = dt                    # free-dim tile for phase A
        self.T = t_dense                # dense chunks (edges per dst) per sec
        self.COV = None                 # overflow chunks per section (data)


def _dtiles(total, dt):
    return [(i, min(dt, total - i)) for i in range(0, total, dt)]


# ---------------------------------------------------------------- phase A
def build_phase_a(cfg):
    nc = bacc.Bacc(num_devices=NCORES)
    D = cfg.DIN
    xT = nc.dram_tensor("xT", [D, cfg.NPC], BF16, kind="ExternalInput")
    posT = nc.dram_tensor("posT", [3, cfg.NPC], BF16, kind="ExternalInput")
    h_w1 = nc.dram_tensor("h_w1", [D, D], BF16, kind="ExternalInput")
    h_b1 = nc.dram_tensor("h_b1", [D, 1], F32, kind="ExternalInput")
    h_w2 = nc.dram_tensor("h_w2", [D, 3], BF16, kind="ExternalInput")
    h_b2 = nc.dram_tensor("h_b2", [3, 1], F32, kind="ExternalInput")
    f_w3 = nc.dram_tensor("f_w3", [3, D], BF16, kind="ExternalInput")
    f_wx = nc.dram_tensor("f_wx", [D, D], BF16, kind="ExternalInput")
    f_b = nc.dram_tensor("f_b", [D, 1], F32, kind="ExternalInput")
    uT = nc.dram_tensor("uT", [D, cfg.NPC], BF16, kind="ExternalOutput")
    vT = nc.dram_tensor("vT", [D, cfg.NPC], BF16, kind="ExternalOutput")

    with tile.TileContext(nc) as tc:
        with (
            tc.tile_pool(name="consts", bufs=1) as cp,
            tc.tile_pool(name="work", bufs=2) as wp,
            tc.tile_pool(name="psum", bufs=2, space="PSUM") as pp,
        ):
            w1_sb = cp.tile([D, D], BF16)
            nc.sync.dma_start(out=w1_sb[:], in_=h_w1[:])
            w2_sb = cp.tile([D, 3], BF16)
            nc.sync.dma_start(out=w2_sb[:], in_=h_w2[:])
            fw3_sb = cp.tile([3, D], BF16)
            nc.sync.dma_start(out=fw3_sb[:], in_=f_w3[:])
            fwx_sb = cp.tile([D, D], BF16)
            nc.sync.dma_start(out=fwx_sb[:], in_=f_wx[:])
            b1_sb = cp.tile([D, 1], F32)
            nc.sync.dma_start(out=b1_sb[:], in_=h_b1[:])
            b2_sb = cp.tile([3, 1], F32)
            nc.sync.dma_start(out=b2_sb[:], in_=h_b2[:])
            fb_sb = cp.tile([D, 1], F32)
            nc.sync.dma_start(out=fb_sb[:], in_=f_b[:])

            for off, w in _dtiles(cfg.NPC, cfg.DT):
                sl = slice(off, off + w)
                xt_t = wp.tile([D, cfg.DT], BF16, tag="xt_t")
                nc.sync.dma_start(out=xt_t[:, :w], in_=xT[:, sl])
                pt_t = wp.tile([3, cfg.DT], BF16, tag="pt_t")
                nc.sync.dma_start(out=pt_t[:, :w], in_=posT[:, sl])
                xt = xt_t[:, :w]
                pt = pt_t[:, :w]
                # t1 = relu(x @ h_w1 + h_b1)   [D, w]
                ps1 = pp.tile([D, cfg.DT], F32, tag="ps1")
                nc.tensor.matmul(out=ps1[:, :w], lhsT=w1_sb[:], rhs=xt,
                                 start=True, stop=True)
                t1 = wp.tile([D, cfg.DT], BF16, tag="t1")
                nc.scalar.activation(out=t1[:, :w], in_=ps1[:, :w], func=AF.Relu,
                                     bias=b1_sb[:])
                # delta = tanh(t1 @ h_w2 + h_b2)  [3, w]
                ps2 = pp.tile([3, cfg.DT], F32, tag="ps2")
                nc.tensor.matmul(out=ps2[:, :w], lhsT=w2_sb[:], rhs=t1[:, :w],
                                 start=True, stop=True)
                dmp = wp.tile([3, cfg.DT], BF16, tag="dmp")
                nc.scalar.activation(out=dmp[:, :w], in_=ps2[:, :w], func=AF.Tanh,
                                     bias=b2_sb[:])
                # diff = delta - pos  [3, w]
                nc.vector.tensor_tensor(out=dmp[:, :w], in0=dmp[:, :w],
                                        in1=pt, op=ALU.subtract)
                # u = pos @ f_w3 + x @ f_wx   [D, w]
                psu = pp.tile([D, cfg.DT], F32, tag="psu")
                nc.tensor.matmul(out=psu[:, :w], lhsT=fw3_sb[:], rhs=pt,
                                 start=True, stop=False)
                nc.tensor.matmul(out=psu[:, :w], lhsT=fwx_sb[:], rhs=xt,
                                 start=False, stop=True)
                ut = wp.tile([D, cfg.DT], BF16, tag="ut")
                nc.vector.tensor_copy(out=ut[:, :w], in_=psu[:, :w])
                nc.sync.dma_start(out=uT[:, sl], in_=ut[:, :w])
                # v = (delta - pos) @ f_w3 + f_b  [D, w]
                psv = pp.tile([D, cfg.DT], F32, tag="psv")
                nc.tensor.matmul(out=psv[:, :w], lhsT=fw3_sb[:], rhs=dmp[:, :w],
                                 start=True, stop=True)
                vt = wp.tile([D, cfg.DT], BF16, tag="vt")
                nc.scalar.activation(out=vt[:, :w], in_=psv[:, :w],
                                     func=AF.Identity, bias=fb_sb[:])
                nc.sync.dma_start(out=vT[:, sl], in_=vt[:, :w])
    nc.finalize()
    return nc


def _fold_pairs(nc, wp, msg4, secs, T, D, tag):
    """Pairwise-sum msg4 [128, SGS, T, D] over the T axis -> [128, SGS, D].

    Returns an AP of shape [128, secs, D]. Emits ceil-tree tensor_tensor
    adds (bf16, packed last dim -> DVE fast mode)."""
    cur = msg4          # AP provider: current level tile, logical width wcur
    wcur = T
    lvl = 0
    while wcur > 1:
        half = wcur // 2
        nxt_w = half + (wcur % 2)
        nxt = wp.tile([128, msg4.shape[1], nxt_w, D], BF16,
                      tag=f"{tag}_l{lvl}")
        nc.vector.tensor_tensor(
            out=nxt[:, :secs, :half, :],
            in0=cur[:, :secs, 0:2 * half:2, :],
            in1=cur[:, :secs, 1:2 * half:2, :],
            op=ALU.add)
        if wcur % 2:
            # carry the odd tail chunk down a level
            nc.vector.tensor_copy(out=nxt[:, :secs, half:half + 1, :],
                                  in_=cur[:, :secs, wcur - 1:wcur, :])
        cur = nxt
        wcur = nxt_w
        lvl += 1
    return cur


# ---------------------------------------------------------------- phase B
def build_phase_b(cfg):
    nc = bacc.Bacc(num_devices=NCORES)
    D = cfg.DIN
    T = cfg.T
    COV = cfg.COV
    NSEC = cfg.NSEC
    SGS = cfg.SG_SECS

    u_d = nc.dram_tensor("u_d", [128, NSEC * T, D], BF16, kind="ExternalInput")
    u_o = nc.dram_tensor("u_o", [128, NSEC * COV, D], BF16, kind="ExternalInput")
    v_o = nc.dram_tensor("v_o", [128, NSEC * COV, D], BF16, kind="ExternalInput")
    vW = nc.dram_tensor("vW", [128, NSEC, D], BF16, kind="ExternalInput")
    pdl = nc.dram_tensor("pdl", [128, NSEC * COV], BF16, kind="ExternalInput")
    xTb = nc.dram_tensor("xTb", [D, cfg.NPC], BF16, kind="ExternalInput")
    gw1 = nc.dram_tensor("gw1", [D, D], BF16, kind="ExternalInput")
    gb1 = nc.dram_tensor("gb1", [D, 1], F32, kind="ExternalInput")
    gw2 = nc.dram_tensor("gw2", [D, D], BF16, kind="ExternalInput")
    gb2 = nc.dram_tensor("gb2", [D, 1], F32, kind="ExternalInput")
    outT = nc.dram_tensor("outT", [D, cfg.NPC], F32, kind="ExternalOutput")

    iota = nc.inline_tensor(
        np.broadcast_to(np.arange(128, dtype=BF), (128, 128)).copy(),
        name="iota")
    ident = nc.inline_tensor(np.eye(128, dtype=BF), name="ident")

    with tile.TileContext(nc) as tc:
        with (
            tc.tile_pool(name="consts", bufs=1) as cp,
            tc.tile_pool(name="stream", bufs=2) as gp,
            tc.tile_pool(name="cwork", bufs=2) as wp,
            tc.tile_pool(name="psagg", bufs=2, space="PSUM") as pa,
            tc.tile_pool(name="psmlp", bufs=2, space="PSUM") as pm,
        ):
            iota_sb = cp.tile([128, 128], BF16)
            nc.sync.dma_start(out=iota_sb[:], in_=iota[:])
            ident_sb = cp.tile([128, 128], BF16)
            nc.sync.dma_start(out=ident_sb[:], in_=ident[:])
            pdl_sb = cp.tile([128, NSEC * COV], BF16)
            nc.sync.dma_start(out=pdl_sb[:], in_=pdl[:])
            gw1_sb = cp.tile([D, D], BF16)
            nc.sync.dma_start(out=gw1_sb[:], in_=gw1[:])
            gw2_sb = cp.tile([D, D], BF16)
            nc.sync.dma_start(out=gw2_sb[:], in_=gw2[:])
            gb1_sb = cp.tile([D, 1], F32)
            nc.sync.dma_start(out=gb1_sb[:], in_=gb1[:])
            gb2_sb = cp.tile([D, 1], F32)
            nc.sync.dma_start(out=gb2_sb[:], in_=gb2[:])

            for sg in range(cfg.NSG):
                s0 = sg * SGS
                s1 = min(s0 + SGS, NSEC)
                secs = s1 - s0

                ue_d = gp.tile([128, SGS, T, D], BF16, tag="ue_d")
                nc.sync.dma_start(
                    out=ue_d[:, :secs, :, :].rearrange("p s r f -> p (s r) f"),
                    in_=u_d[:, s0 * T:s1 * T, :])
                v_sg = gp.tile([128, SGS, D], BF16, tag="v_sg")
                nc.sync.dma_start(out=v_sg[:, :secs, :], in_=vW[:, s0:s1, :])
                ue_o = gp.tile([128, SGS * COV, D], BF16, tag="ue_o")
                nc.sync.dma_start(out=ue_o[:, :secs * COV, :],
                                  in_=u_o[:, s0 * COV:s1 * COV, :])
                ve_o = gp.tile([128, SGS * COV, D], BF16, tag="ve_o")
                nc.sync.dma_start(out=ve_o[:, :secs * COV, :],
                                  in_=v_o[:, s0 * COV:s1 * COV, :])

                # dense: msg = relu(u + v_sec)   [p, s, r, f]
                msg_d = wp.tile([128, SGS, T, D], BF16, tag="msg_d")
                nc.vector.tensor_tensor(
                    out=msg_d[:, :secs, :, :],
                    in0=ue_d[:, :secs, :, :],
                    in1=v_sg[:, :secs, None, :].to_broadcast([128, secs, T, D]),
                    op=ALU.add)
                mdf = msg_d[:, :secs, :, :].rearrange("p s r f -> p (s r f)")
                nc.vector.tensor_relu(mdf, mdf)
                dsum = _fold_pairs(nc, wp, msg_d, secs, T, D, tag="fold")

                # overflow: msg = relu(u + v)
                msg_o = wp.tile([128, SGS * COV, D], BF16, tag="msg_o")
                mof = msg_o[:, :secs * COV, :].rearrange("p c f -> p (c f)")
                nc.vector.tensor_tensor(
                    out=mof,
                    in0=ue_o[:, :secs * COV, :].rearrange("p c f -> p (c f)"),
                    in1=ve_o[:, :secs * COV, :].rearrange("p c f -> p (c f)"),
                    op=ALU.add)
                nc.vector.tensor_relu(mof, mof)

                # overflow selection matrices: S[p, c, w] = (pdl==w)
                st = wp.tile([128, SGS * COV, 128], BF16, tag="st")
                nc.vector.tensor_tensor(
                    out=st[:, :secs * COV, :],
                    in0=pdl_sb[:, s0 * COV:s1 * COV, None]
                        .to_broadcast([128, secs * COV, 128]),
                    in1=iota_sb[:, None, :]
                        .to_broadcast([128, secs * COV, 128]),
                    op=ALU.is_equal)

                # segment-sum into psum [feat, w]
                ps = pa.tile([D, SGS * cfg.SEC], F32, tag="psagg")
                for j in range(secs):
                    osl = slice(j * cfg.SEC, (j + 1) * cfg.SEC)
                    nc.tensor.matmul(out=ps[:, osl], lhsT=dsum[:, j, :],
                                     rhs=ident_sb[:], start=True,
                                     stop=(COV == 0))
                    for t in range(COV):
                        nc.tensor.matmul(
                            out=ps[:, osl],
                            lhsT=msg_o[:, j * COV + t, :],
                            rhs=st[:, j * COV + t, :],
                            start=False, stop=(t == COV - 1))
                aggt = wp.tile([D, SGS * cfg.SEC], BF16, tag="aggt")
                nc.scalar.activation(out=aggt[:, :secs * cfg.SEC],
                                     in_=ps[:, :secs * cfg.SEC], func=AF.Copy)

                # fused tail: out = x + relu(relu(agg@g_w1+g_b1)@g_w2+g_b2)
                n0 = s0 * cfg.SEC
                w = min(cfg.NPC, s1 * cfg.SEC) - n0
                nsl = slice(n0, n0 + w)
                ph1 = pm.tile([D, SGS * cfg.SEC], F32, tag="ph1")
                nc.tensor.matmul(out=ph1[:, :w], lhsT=gw1_sb[:],
                                 rhs=aggt[:, :w], start=True, stop=True)
                h1 = wp.tile([D, SGS * cfg.SEC], BF16, tag="h1")
                nc.scalar.activation(out=h1[:, :w], in_=ph1[:, :w], func=AF.Relu,
                                     bias=gb1_sb[:])
                ph2 = pm.tile([D, SGS * cfg.SEC], F32, tag="ph2")
                nc.tensor.matmul(out=ph2[:, :w], lhsT=gw2_sb[:],
                                 rhs=h1[:, :w], start=True, stop=True)
                h2 = wp.tile([D, SGS * cfg.SEC], F32, tag="h2")
                nc.scalar.activation(out=h2[:, :w], in_=ph2[:, :w], func=AF.Relu,
                                     bias=gb2_sb[:])
                xt = wp.tile([D, SGS * cfg.SEC], BF16, tag="xt")
                nc.sync.dma_start(out=xt[:, :w], in_=xTb[:, nsl])
                nc.vector.tensor_tensor(out=h2[:, :w], in0=h2[:, :w],
                                        in1=xt[:, :w], op=ALU.add)
                nc.sync.dma_start(out=outT[:, nsl], in_=h2[:, :w])
    nc.finalize()
    return nc


# ------------------------------------------------------------ host side
def _preprocess(cfg, edge_index):
    """Sort edges by dst per core; dense/overflow slot assignment.

    Sets cfg.COV. Returns per-core dict with:
      idx_dense [NSEC*T*128] int64  (src node id per dense slot, -1 pad)
      idx_osrc  [NSEC*COV*128] int64 (src per overflow slot, -1 pad)
      idx_odst  [NSEC*COV*128] int64 (core-local dst per ov slot, -1 pad)
      pdl_w [128, NSEC*COV] bf16 (dst%128 per ov slot, -1 pad)
    """
    src = np.asarray(edge_index[0], dtype=np.int64)
    dst = np.asarray(edge_index[1], dtype=np.int64)
    order = np.argsort(dst, kind="stable")
    src, dst = src[order], dst[order]
    core = dst // cfg.NPC
    bounds = np.searchsorted(core, np.arange(NCORES + 1))
    T = cfg.T

    percore = []
    cov_max = 1
    for c in range(NCORES):
        lo, hi = bounds[c], bounds[c + 1]
        s, d = src[lo:hi], dst[lo:hi] - c * cfg.NPC
        deg = np.bincount(d, minlength=cfg.NPC)
        first = np.zeros(cfg.NPC, np.int64)
        np.cumsum(deg[:-1], out=first[1:])
        rank = np.arange(len(d)) - first[d]
        sec = d >> 7
        exc = np.maximum(deg - T, 0)
        exc_pad = np.zeros(cfg.NSEC * cfg.SEC, np.int64)
        exc_pad[:cfg.NPC] = exc
        sec_exc = exc_pad.reshape(cfg.NSEC, cfg.SEC).sum(1)
        cov_max = max(cov_max, int(np.ceil(sec_exc.max() / 128)))
        percore.append((s, d, sec, rank))
    cfg.COV = cov_max
    COV = cov_max

    out = []
    for c in range(NCORES):
        s, d, sec, rank = percore[c]
        md = rank < T
        idx_dense = np.full(cfg.NSEC * T * 128, -1, np.int64)
        slot_d = (sec[md] * T + rank[md]) * 128 + (d[md] & 127)
        idx_dense[slot_d] = s[md]

        mo = ~md
        sec_o = sec[mo]
        ostart = np.zeros(cfg.NSEC, np.int64)
        cnt_o = np.bincount(sec_o, minlength=cfg.NSEC)
        np.cumsum(cnt_o[:-1], out=ostart[1:])
        q = np.arange(len(sec_o)) - ostart[sec_o]
        slot_o = (sec_o * COV + (q >> 7)) * 128 + (q & 127)
        idx_osrc = np.full(cfg.NSEC * COV * 128, -1, np.int64)
        idx_odst = np.full(cfg.NSEC * COV * 128, -1, np.int64)
        idx_osrc[slot_o] = s[mo]
        idx_odst[slot_o] = d[mo]
        pdl_flat = np.full(cfg.NSEC * COV * 128, -1.0, np.float32)
        pdl_flat[slot_o] = (d[mo] & 127).astype(np.float32)
        pdl_w = np.ascontiguousarray(
            pdl_flat.reshape(-1, 128).T).astype(BF)
        out.append({"idx_dense": idx_dense, "idx_osrc": idx_osrc,
                    "idx_odst": idx_odst, "pdl_w": pdl_w})
    return out


def _expand(tbl, idx, ncols):
    """Gather rows of tbl by idx (zero row for idx<0), wrap to [128,ncols,D]."""
    rows = np.zeros((len(idx), tbl.shape[1]), dtype=tbl.dtype)
    valid = idx >= 0
    rows[valid] = tbl[idx[valid]]
    return np.ascontiguousarray(
        rows.reshape(ncols, 128, -1).transpose(1, 0, 2))


def run(cfg, inputs, trace=False):
    """Full pipeline. inputs: dict as from setup_inputs (numpy)."""
    x = np.asarray(inputs["x"], np.float32)
    pos = np.asarray(inputs["pos"], np.float32)
    edata = _preprocess(cfg, np.asarray(inputs["edge_index"]))

    h_w1 = np.asarray(inputs["h_w1"], np.float32)
    h_b1 = np.asarray(inputs["h_b1"], np.float32)
    h_w2 = np.asarray(inputs["h_w2"], np.float32)
    h_b2 = np.asarray(inputs["h_b2"], np.float32)
    f_w = np.asarray(inputs["f_w"], np.float32)
    f_b = np.asarray(inputs["f_b"], np.float32)
    g_w1 = np.asarray(inputs["g_w1"], np.float32)
    g_b1 = np.asarray(inputs["g_b1"], np.float32)
    g_w2 = np.asarray(inputs["g_w2"], np.float32)
    g_b2 = np.asarray(inputs["g_b2"], np.float32)

    nc_a = build_phase_a(cfg)
    in_a = []
    for c in range(NCORES):
        sl = slice(c * cfg.NPC, (c + 1) * cfg.NPC)
        in_a.append({
            "xT": np.ascontiguousarray(x[sl].T.astype(BF)),
            "posT": np.ascontiguousarray(pos[sl].T.astype(BF)),
            "h_w1": h_w1.astype(BF), "h_b1": h_b1[:, None],
            "h_w2": h_w2.astype(BF), "h_b2": h_b2[:, None],
            "f_w3": f_w[:3].astype(BF), "f_wx": f_w[3:].astype(BF),
            "f_b": f_b[:, None],
        })
    res_a = run_bass_kernel_spmd(nc_a, in_a, core_ids=list(range(NCORES)),
                                 trace=trace)
    # u table node-major over ALL nodes; v tables per-core node-major
    u_nm = np.concatenate(
        [np.ascontiguousarray(np.asarray(r["uT"]).T) for r in res_a.results],
        axis=0)
    v_nms = [np.ascontiguousarray(np.asarray(r["vT"]).T) for r in res_a.results]

    nc_b = build_phase_b(cfg)
    T, COV = cfg.T, cfg.COV
    in_b = []
    for c in range(NCORES):
        sl = slice(c * cfg.NPC, (c + 1) * cfg.NPC)
        ed = edata[c]
        v_nm = v_nms[c]
        # vW [128, NSEC, D]: vW[p, s] = v[s*128+p] (zero-pad past NPC)
        vpad = np.zeros((cfg.NSEC * cfg.SEC, cfg.DIN), dtype=v_nm.dtype)
        vpad[:cfg.NPC] = v_nm
        vW = np.ascontiguousarray(
            vpad.reshape(cfg.NSEC, 128, cfg.DIN).transpose(1, 0, 2))
        in_b.append({
            "u_d": _expand(u_nm, ed["idx_dense"], cfg.NSEC * T),
            "u_o": _expand(u_nm, ed["idx_osrc"], cfg.NSEC * COV),
            "v_o": _expand(v_nm, ed["idx_odst"], cfg.NSEC * COV),
            "vW": vW,
            "pdl": ed["pdl_w"],
            "xTb": np.ascontiguousarray(x[sl].T.astype(BF)),
            "gw1": g_w1.astype(BF), "gb1": g_b1[:, None],
            "gw2": g_w2.astype(BF), "gb2": g_b2[:, None],
        })
    res_b = run_bass_kernel_spmd(nc_b, in_b, core_ids=list(range(NCORES)),
                                 trace=trace)
    out = np.concatenate(
        [np.ascontiguousarray(np.asarray(r["outT"]).T) for r in res_b.results],
        axis=0)
    return out, (res_a, res_b)


DEFAULT_CFG = Cfg(n=50000, e=500000, din=128)


def kernel(**inputs):
    out, _ = run(DEFAULT_CFG, inputs)
    return out.astype(np.float32)


# revision 13
# speedup vs baseline: 6.7449x; 1.0370x over previous
"""PointGNNConv (sum aggregation) on 8 Trainium2 NeuronCores.

Algebraic decomposition: with f_w = [f_w3; f_wx] (3+128 rows),
    msg_e = relu(edge_feat @ f_w + f_b) = relu(u[src_e] + v[dst_e])
    u_j = pos_j @ f_w3 + x_j @ f_wx
    v_i = (delta_i - pos_i) @ f_w3 + f_b

Sharding: dst-range sharding -- core c owns dst in [c*NPC, (c+1)*NPC).

Two NEFFs. Phase A computes per-node u/v (bf16) on each core's node slice.
Between NEFFs the host expands the tables into per-edge streams (pure row
gather / reordering, no FP) so phase B needs NO on-device gather (the SWDGE
per-edge gather of the original design serialized ~1.2ms of descriptor
generation on GpSimd).

Phase B edge layout (per core, edges sorted by dst, sections of 128 dsts):
 - DENSE: the first T edges of each dst go to column-aligned chunks -- slot
   p of dense chunk r holds the r-th edge of dst (sec_base+p). The add of
   v[dst] uses the *unexpanded* per-section v tile broadcast across chunks
   (no v stream), and the segment-sum over chunks is a DVE pairwise tree
   followed by ONE identity matmul per section (psum transpose-accumulate).
 - OVERFLOW: edges beyond T per dst (25% here) go to packed chunks with a
   dst-local label (pdl); selection matrices S[slot,w] = (pdl[slot]==w) are
   built on GpSimd (is_equal) and matmul-accumulated into the same psum
   window.
Everything is bf16 (DVE 2x/4x fast modes need 2-byte packed operands; fp8
runs at base rate), psum f32, output f32. The g-MLP + residual tail is
fused per supergroup (4 sections / 512 dsts).
"""
import sys

sys.path.insert(0, "/opt/trn_rl_repo")

import numpy as np
import ml_dtypes

import concourse.bass as bass
import concourse.mybir as mybir
import concourse.tile as tile
from concourse import bacc
from concourse.bass_utils import run_bass_kernel_spmd

BF16 = mybir.dt.bfloat16
F32 = mybir.dt.float32
AF = mybir.ActivationFunctionType
ALU = mybir.AluOpType

NCORES = 8
BF = ml_dtypes.bfloat16


class Cfg:
    def __init__(self, n, e, din, dt=512, t_dense=7):
        self.N = n
        self.E = e
        self.DIN = din
        self.NPC = n // NCORES          # nodes (dsts) per core
        self.SEC = 128                  # dsts per section
        self.NSEC = -(-self.NPC // self.SEC)
        self.SG_SECS = 4                # sections per supergroup (psum window)
        self.NSG = -(-self.NSEC // self.SG_SECS)
        self.DT = dt                    # free-dim tile for phase A
        self.T = t_dense                # dense chunks (edges per dst) per sec
        self.COV = None                 # overflow chunks per section (data)


def _dtiles(total, dt):
    return [(i, min(dt, total - i)) for i in range(0, total, dt)]


# ---------------------------------------------------------------- phase A
def build_phase_a(cfg):
    nc = bacc.Bacc(num_devices=NCORES)
    D = cfg.DIN
    xT = nc.dram_tensor("xT", [D, cfg.NPC], BF16, kind="ExternalInput")
    posT = nc.dram_tensor("posT", [3, cfg.NPC], BF16, kind="ExternalInput")
    h_w1 = nc.dram_tensor("h_w1", [D, D], BF16, kind="ExternalInput")
    h_b1 = nc.dram_tensor("h_b1", [D, 1], F32, kind="ExternalInput")
    h_w2 = nc.dram_tensor("h_w2", [D, 3], BF16, kind="ExternalInput")
    h_b2 = nc.dram_tensor("h_b2", [3, 1], F32, kind="ExternalInput")
    f_w3 = nc.dram_tensor("f_w3", [3, D], BF16, kind="ExternalInput")
    f_wx = nc.dram_tensor("f_wx", [D, D], BF16, kind="ExternalInput")
    f_b = nc.dram_tensor("f_b", [D, 1], F32, kind="ExternalInput")
    uT = nc.dram_tensor("uT", [D, cfg.NPC], BF16, kind="ExternalOutput")
    vT = nc.dram_tensor("vT", [D, cfg.NPC], BF16, kind="ExternalOutput")

    with tile.TileContext(nc) as tc:
        with (
            tc.tile_pool(name="consts", bufs=1) as cp,
            tc.tile_pool(name="work", bufs=2) as wp,
            tc.tile_pool(name="psum", bufs=2, space="PSUM") as pp,
        ):
            w1_sb = cp.tile([D, D], BF16)
            nc.sync.dma_start(out=w1_sb[:], in_=h_w1[:])
            w2_sb = cp.tile([D, 3], BF16)
            nc.sync.dma_start(out=w2_sb[:], in_=h_w2[:])
            fw3_sb = cp.tile([3, D], BF16)
            nc.sync.dma_start(out=fw3_sb[:], in_=f_w3[:])
            fwx_sb = cp.tile([D, D], BF16)
            nc.sync.dma_start(out=fwx_sb[:], in_=f_wx[:])
            b1_sb = cp.tile([D, 1], F32)
            nc.sync.dma_start(out=b1_sb[:], in_=h_b1[:])
            b2_sb = cp.tile([3, 1], F32)
            nc.sync.dma_start(out=b2_sb[:], in_=h_b2[:])
            fb_sb = cp.tile([D, 1], F32)
            nc.sync.dma_start(out=fb_sb[:], in_=f_b[:])

            for off, w in _dtiles(cfg.NPC, cfg.DT):
                sl = slice(off, off + w)
                xt_t = wp.tile([D, cfg.DT], BF16, tag="xt_t")
                nc.gpsimd.dma_start(out=xt_t[:, :w], in_=xT[:, sl])
                pt_t = wp.tile([3, cfg.DT], BF16, tag="pt_t")
                nc.gpsimd.dma_start(out=pt_t[:, :w], in_=posT[:, sl])
                xt = xt_t[:, :w]
                pt = pt_t[:, :w]
                # t1 = relu(x @ h_w1 + h_b1)   [D, w]
                ps1 = pp.tile([D, cfg.DT], F32, tag="ps1")
                nc.tensor.matmul(out=ps1[:, :w], lhsT=w1_sb[:], rhs=xt,
                                 start=True, stop=True)
                t1 = wp.tile([D, cfg.DT], BF16, tag="t1")
                nc.scalar.activation(out=t1[:, :w], in_=ps1[:, :w], func=AF.Relu,
                                     bias=b1_sb[:])
                # delta = tanh(t1 @ h_w2 + h_b2)  [3, w]
                ps2 = pp.tile([3, cfg.DT], F32, tag="ps2")
                nc.tensor.matmul(out=ps2[:, :w], lhsT=w2_sb[:], rhs=t1[:, :w],
                                 start=True, stop=True)
                dmp = wp.tile([3, cfg.DT], BF16, tag="dmp")
                nc.scalar.activation(out=dmp[:, :w], in_=ps2[:, :w], func=AF.Tanh,
                                     bias=b2_sb[:])
                # diff = delta - pos  [3, w]
                nc.vector.tensor_tensor(out=dmp[:, :w], in0=dmp[:, :w],
                                        in1=pt, op=ALU.subtract)
                # u = pos @ f_w3 + x @ f_wx   [D, w]
                psu = pp.tile([D, cfg.DT], F32, tag="psu")
                nc.tensor.matmul(out=psu[:, :w], lhsT=fw3_sb[:], rhs=pt,
                                 start=True, stop=False)
                nc.tensor.matmul(out=psu[:, :w], lhsT=fwx_sb[:], rhs=xt,
                                 start=False, stop=True)
                ut = wp.tile([D, cfg.DT], BF16, tag="ut")
                nc.vector.tensor_copy(out=ut[:, :w], in_=psu[:, :w])
                nc.sync.dma_start(out=uT[:, sl], in_=ut[:, :w])
                # v = (delta - pos) @ f_w3 + f_b  [D, w]
                psv = pp.tile([D, cfg.DT], F32, tag="psv")
                nc.tensor.matmul(out=psv[:, :w], lhsT=fw3_sb[:], rhs=dmp[:, :w],
                                 start=True, stop=True)
                vt = wp.tile([D, cfg.DT], BF16, tag="vt")
                nc.vector.tensor_scalar_add(out=vt[:, :w], in0=psv[:, :w],
                                            scalar1=fb_sb[:])
                nc.sync.dma_start(out=vT[:, sl], in_=vt[:, :w])
    nc.finalize()
    return nc


def _fold_pairs(nc, wp, msg4, secs, T, D, tag):
    """Pairwise-sum msg4 [128, SGS, T, D] over the T axis -> [128, SGS, D].

    Returns an AP of shape [128, secs, D]. Emits ceil-tree tensor_tensor
    adds (bf16, packed last dim -> DVE fast mode)."""
    cur = msg4          # AP provider: current level tile, logical width wcur
    wcur = T
    lvl = 0
    while wcur > 1:
        half = wcur // 2
        nxt_w = half + (wcur % 2)
        nxt = wp.tile([128, msg4.shape[1], nxt_w, D], BF16,
                      tag=f"{tag}_l{lvl}")
        nc.vector.tensor_tensor(
            out=nxt[:, :secs, :half, :],
            in0=cur[:, :secs, 0:2 * half:2, :],
            in1=cur[:, :secs, 1:2 * half:2, :],
            op=ALU.add)
        if wcur % 2:
            # carry the odd tail chunk down a level
            nc.vector.tensor_copy(out=nxt[:, :secs, half:half + 1, :],
                                  in_=cur[:, :secs, wcur - 1:wcur, :])
        cur = nxt
        wcur = nxt_w
        lvl += 1
    return cur


# ---------------------------------------------------------------- phase B
def build_phase_b(cfg):
    nc = bacc.Bacc(num_devices=NCORES)
    D = cfg.DIN
    T = cfg.T
    COV = cfg.COV
    NSEC = cfg.NSEC
    SGS = cfg.SG_SECS

    u_d = nc.dram_tensor("u_d", [128, NSEC * T, D], BF16, kind="ExternalInput")
    u_o = nc.dram_tensor("u_o", [128, NSEC * COV, D], BF16, kind="ExternalInput")
    v_o = nc.dram_tensor("v_o", [128, NSEC * COV, D], BF16, kind="ExternalInput")
    vW = nc.dram_tensor("vW", [128, NSEC, D], BF16, kind="ExternalInput")
    pdl = nc.dram_tensor("pdl", [128, NSEC * COV], BF16, kind="ExternalInput")
    xTb = nc.dram_tensor("xTb", [D, cfg.NPC], BF16, kind="ExternalInput")
    gw1 = nc.dram_tensor("gw1", [D, D], BF16, kind="ExternalInput")
    gb1 = nc.dram_tensor("gb1", [D, 1], F32, kind="ExternalInput")
    gw2 = nc.dram_tensor("gw2", [D, D], BF16, kind="ExternalInput")
    gb2 = nc.dram_tensor("gb2", [D, 1], F32, kind="ExternalInput")
    outT = nc.dram_tensor("outT", [D, cfg.NPC], BF16, kind="ExternalOutput")

    iota = nc.inline_tensor(
        np.broadcast_to(np.arange(128, dtype=BF), (128, 128)).copy(),
        name="iota")
    ident = nc.inline_tensor(np.eye(128, dtype=BF), name="ident")

    with tile.TileContext(nc) as tc:
        with (
            tc.tile_pool(name="consts", bufs=1) as cp,
            tc.tile_pool(name="stream", bufs=2) as gp,
            tc.tile_pool(name="cwork", bufs=2) as wp,
            tc.tile_pool(name="psagg", bufs=2, space="PSUM") as pa,
            tc.tile_pool(name="psmlp", bufs=2, space="PSUM") as pm,
        ):
            iota_sb = cp.tile([128, 128], BF16)
            nc.sync.dma_start(out=iota_sb[:], in_=iota[:])
            ident_sb = cp.tile([128, 128], BF16)
            nc.sync.dma_start(out=ident_sb[:], in_=ident[:])
            pdl_sb = cp.tile([128, NSEC * COV], BF16)
            nc.sync.dma_start(out=pdl_sb[:], in_=pdl[:])
            gw1_sb = cp.tile([D, D], BF16)
            nc.sync.dma_start(out=gw1_sb[:], in_=gw1[:])
            gw2_sb = cp.tile([D, D], BF16)
            nc.sync.dma_start(out=gw2_sb[:], in_=gw2[:])
            gb1_sb = cp.tile([D, 1], F32)
            nc.sync.dma_start(out=gb1_sb[:], in_=gb1[:])
            gb2_sb = cp.tile([D, 1], F32)
            nc.sync.dma_start(out=gb2_sb[:], in_=gb2[:])

            for sg in range(cfg.NSG):
                s0 = sg * SGS
                s1 = min(s0 + SGS, NSEC)
                secs = s1 - s0

                ue_d = gp.tile([128, SGS, T, D], BF16, tag="ue_d")
                nc.gpsimd.dma_start(
                    out=ue_d[:, :secs, :, :].rearrange("p s r f -> p (s r) f"),
                    in_=u_d[:, s0 * T:s1 * T, :])
                v_sg = gp.tile([128, SGS, D], BF16, tag="v_sg")
                nc.gpsimd.dma_start(out=v_sg[:, :secs, :], in_=vW[:, s0:s1, :])
                ue_o = gp.tile([128, SGS * COV, D], BF16, tag="ue_o")
                nc.gpsimd.dma_start(out=ue_o[:, :secs * COV, :],
                                    in_=u_o[:, s0 * COV:s1 * COV, :])
                ve_o = gp.tile([128, SGS * COV, D], BF16, tag="ve_o")
                nc.gpsimd.dma_start(out=ve_o[:, :secs * COV, :],
                                    in_=v_o[:, s0 * COV:s1 * COV, :])

                # dense: msg = relu(u + v_sec)   [p, s, r, f]
                msg_d = wp.tile([128, SGS, T, D], BF16, tag="msg_d")
                nc.vector.tensor_tensor(
                    out=msg_d[:, :secs, :, :],
                    in0=ue_d[:, :secs, :, :],
                    in1=v_sg[:, :secs, None, :].to_broadcast([128, secs, T, D]),
                    op=ALU.add)
                mdf = msg_d[:, :secs, :, :].rearrange("p s r f -> p (s r f)")
                nc.vector.tensor_relu(mdf, mdf)
                # one pairwise-fold level on DVE; the T//2 (+carry) partial
                # sums then go through identity matmuls (psum accumulate)
                npair = T // 2
                r1 = None
                if npair:
                    r1 = wp.tile([128, SGS, npair, D], BF16, tag="r1")
                    nc.vector.tensor_tensor(
                        out=r1[:, :secs, :, :],
                        in0=msg_d[:, :secs, 0:2 * npair:2, :],
                        in1=msg_d[:, :secs, 1:2 * npair:2, :],
                        op=ALU.add)

                # overflow: msg = relu(u + v)
                msg_o = wp.tile([128, SGS * COV, D], BF16, tag="msg_o")
                mof = msg_o[:, :secs * COV, :].rearrange("p c f -> p (c f)")
                nc.vector.tensor_tensor(
                    out=mof,
                    in0=ue_o[:, :secs * COV, :].rearrange("p c f -> p (c f)"),
                    in1=ve_o[:, :secs * COV, :].rearrange("p c f -> p (c f)"),
                    op=ALU.add)
                nc.vector.tensor_relu(mof, mof)

                # overflow selection matrices: S[p, c, w] = (pdl==w)
                st = wp.tile([128, SGS * COV, 128], BF16, tag="st")
                nc.vector.tensor_tensor(
                    out=st[:, :secs * COV, :],
                    in0=pdl_sb[:, s0 * COV:s1 * COV, None]
                        .to_broadcast([128, secs * COV, 128]),
                    in1=iota_sb[:, None, :]
                        .to_broadcast([128, secs * COV, 128]),
                    op=ALU.is_equal)

                # segment-sum into psum [feat, w]
                ps = pa.tile([D, SGS * cfg.SEC], F32, tag="psagg")
                for j in range(secs):
                    osl = slice(j * cfg.SEC, (j + 1) * cfg.SEC)
                    idchunks = [r1[:, j, t, :] for t in range(npair)]
                    if T % 2:
                        idchunks.append(msg_d[:, j, T - 1, :])
                    for t, ch in enumerate(idchunks):
                        nc.tensor.matmul(out=ps[:, osl], lhsT=ch,
                                         rhs=ident_sb[:], start=(t == 0),
                                         stop=(COV == 0 and
                                               t == len(idchunks) - 1))
                    for t in range(COV):
                        nc.tensor.matmul(
                            out=ps[:, osl],
                            lhsT=msg_o[:, j * COV + t, :],
                            rhs=st[:, j * COV + t, :],
                            start=False, stop=(t == COV - 1))
                aggt = wp.tile([D, SGS * cfg.SEC], BF16, tag="aggt")
                nc.scalar.activation(out=aggt[:, :secs * cfg.SEC],
                                     in_=ps[:, :secs * cfg.SEC], func=AF.Copy)

                # fused tail: out = x + relu(relu(agg@g_w1+g_b1)@g_w2+g_b2)
                n0 = s0 * cfg.SEC
                w = min(cfg.NPC, s1 * cfg.SEC) - n0
                nsl = slice(n0, n0 + w)
                ph1 = pm.tile([D, SGS * cfg.SEC], F32, tag="ph1")
                nc.tensor.matmul(out=ph1[:, :w], lhsT=gw1_sb[:],
                                 rhs=aggt[:, :w], start=True, stop=True)
                h1 = wp.tile([D, SGS * cfg.SEC], BF16, tag="h1")
                nc.scalar.activation(out=h1[:, :w], in_=ph1[:, :w], func=AF.Relu,
                                     bias=gb1_sb[:])
                ph2 = pm.tile([D, SGS * cfg.SEC], F32, tag="ph2")
                nc.tensor.matmul(out=ph2[:, :w], lhsT=gw2_sb[:],
                                 rhs=h1[:, :w], start=True, stop=True)
                h2 = wp.tile([D, SGS * cfg.SEC], F32, tag="h2")
                nc.scalar.activation(out=h2[:, :w], in_=ph2[:, :w], func=AF.Relu,
                                     bias=gb2_sb[:])
                xt = wp.tile([D, SGS * cfg.SEC], BF16, tag="xt")
                nc.sync.dma_start(out=xt[:, :w], in_=xTb[:, nsl])
                ob = wp.tile([D, SGS * cfg.SEC], BF16, tag="ob")
                nc.vector.tensor_tensor(out=ob[:, :w], in0=h2[:, :w],
                                        in1=xt[:, :w], op=ALU.add)
                nc.sync.dma_start(out=outT[:, nsl], in_=ob[:, :w])
    nc.finalize()
    return nc


# ------------------------------------------------------------ host side
def _preprocess(cfg, edge_index):
    """Sort edges by dst per core; dense/overflow slot assignment.

    Sets cfg.COV. Returns per-core dict with:
      idx_dense [NSEC*T*128] int64  (src node id per dense slot, -1 pad)
      idx_osrc  [NSEC*COV*128] int64 (src per overflow slot, -1 pad)
      idx_odst  [NSEC*COV*128] int64 (core-local dst per ov slot, -1 pad)
      pdl_w [128, NSEC*COV] bf16 (dst%128 per ov slot, -1 pad)
    """
    src = np.asarray(edge_index[0], dtype=np.int64)
    dst = np.asarray(edge_index[1], dtype=np.int64)
    order = np.argsort(dst, kind="stable")
    src, dst = src[order], dst[order]
    core = dst // cfg.NPC
    bounds = np.searchsorted(core, np.arange(NCORES + 1))
    T = cfg.T

    percore = []
    cov_max = 1
    for c in range(NCORES):
        lo, hi = bounds[c], bounds[c + 1]
        s, d = src[lo:hi], dst[lo:hi] - c * cfg.NPC
        deg = np.bincount(d, minlength=cfg.NPC)
        first = np.zeros(cfg.NPC, np.int64)
        np.cumsum(deg[:-1], out=first[1:])
        rank = np.arange(len(d)) - first[d]
        sec = d >> 7
        exc = np.maximum(deg - T, 0)
        exc_pad = np.zeros(cfg.NSEC * cfg.SEC, np.int64)
        exc_pad[:cfg.NPC] = exc
        sec_exc = exc_pad.reshape(cfg.NSEC, cfg.SEC).sum(1)
        cov_max = max(cov_max, int(np.ceil(sec_exc.max() / 128)))
        percore.append((s, d, sec, rank))
    cfg.COV = cov_max
    COV = cov_max

    out = []
    for c in range(NCORES):
        s, d, sec, rank = percore[c]
        md = rank < T
        idx_dense = np.full(cfg.NSEC * T * 128, -1, np.int64)
        slot_d = (sec[md] * T + rank[md]) * 128 + (d[md] & 127)
        idx_dense[slot_d] = s[md]

        mo = ~md
        sec_o = sec[mo]
        ostart = np.zeros(cfg.NSEC, np.int64)
        cnt_o = np.bincount(sec_o, minlength=cfg.NSEC)
        np.cumsum(cnt_o[:-1], out=ostart[1:])
        q = np.arange(len(sec_o)) - ostart[sec_o]
        slot_o = (sec_o * COV + (q >> 7)) * 128 + (q & 127)
        idx_osrc = np.full(cfg.NSEC * COV * 128, -1, np.int64)
        idx_odst = np.full(cfg.NSEC * COV * 128, -1, np.int64)
        idx_osrc[slot_o] = s[mo]
        idx_odst[slot_o] = d[mo]
        pdl_flat = np.full(cfg.NSEC * COV * 128, -1.0, np.float32)
        pdl_flat[slot_o] = (d[mo] & 127).astype(np.float32)
        pdl_w = np.ascontiguousarray(
            pdl_flat.reshape(-1, 128).T).astype(BF)
        out.append({"idx_dense": idx_dense, "idx_osrc": idx_osrc,
                    "idx_odst": idx_odst, "pdl_w": pdl_w})
    return out


def _expand(tbl, idx, ncols):
    """Gather rows of tbl by idx (zero row for idx<0), wrap to [128,ncols,D]."""
    rows = np.zeros((len(idx), tbl.shape[1]), dtype=tbl.dtype)
    valid = idx >= 0
    rows[valid] = tbl[idx[valid]]
    return np.ascontiguousarray(
        rows.reshape(ncols, 128, -1).transpose(1, 0, 2))


def run(cfg, inputs, trace=False):
    """Full pipeline. inputs: dict as from setup_inputs (numpy)."""
    x = np.asarray(inputs["x"], np.float32)
    pos = np.asarray(inputs["pos"], np.float32)
    edata = _preprocess(cfg, np.asarray(inputs["edge_index"]))

    h_w1 = np.asarray(inputs["h_w1"], np.float32)
    h_b1 = np.asarray(inputs["h_b1"], np.float32)
    h_w2 = np.asarray(inputs["h_w2"], np.float32)
    h_b2 = np.asarray(inputs["h_b2"], np.float32)
    f_w = np.asarray(inputs["f_w"], np.float32)
    f_b = np.asarray(inputs["f_b"], np.float32)
    g_w1 = np.asarray(inputs["g_w1"], np.float32)
    g_b1 = np.asarray(inputs["g_b1"], np.float32)
    g_w2 = np.asarray(inputs["g_w2"], np.float32)
    g_b2 = np.asarray(inputs["g_b2"], np.float32)

    nc_a = build_phase_a(cfg)
    in_a = []
    for c in range(NCORES):
        sl = slice(c * cfg.NPC, (c + 1) * cfg.NPC)
        in_a.append({
            "xT": np.ascontiguousarray(x[sl].T.astype(BF)),
            "posT": np.ascontiguousarray(pos[sl].T.astype(BF)),
            "h_w1": h_w1.astype(BF), "h_b1": h_b1[:, None],
            "h_w2": h_w2.astype(BF), "h_b2": h_b2[:, None],
            "f_w3": f_w[:3].astype(BF), "f_wx": f_w[3:].astype(BF),
            "f_b": f_b[:, None],
        })
    res_a = run_bass_kernel_spmd(nc_a, in_a, core_ids=list(range(NCORES)),
                                 trace=trace)
    # u table node-major over ALL nodes; v tables per-core node-major
    u_nm = np.concatenate(
        [np.ascontiguousarray(np.asarray(r["uT"]).T) for r in res_a.results],
        axis=0)
    v_nms = [np.ascontiguousarray(np.asarray(r["vT"]).T) for r in res_a.results]

    nc_b = build_phase_b(cfg)
    T, COV = cfg.T, cfg.COV
    in_b = []
    for c in range(NCORES):
        sl = slice(c * cfg.NPC, (c + 1) * cfg.NPC)
        ed = edata[c]
        v_nm = v_nms[c]
        # vW [128, NSEC, D]: vW[p, s] = v[s*128+p] (zero-pad past NPC)
        vpad = np.zeros((cfg.NSEC * cfg.SEC, cfg.DIN), dtype=v_nm.dtype)
        vpad[:cfg.NPC] = v_nm
        vW = np.ascontiguousarray(
            vpad.reshape(cfg.NSEC, 128, cfg.DIN).transpose(1, 0, 2))
        in_b.append({
            "u_d": _expand(u_nm, ed["idx_dense"], cfg.NSEC * T),
            "u_o": _expand(u_nm, ed["idx_osrc"], cfg.NSEC * COV),
            "v_o": _expand(v_nm, ed["idx_odst"], cfg.NSEC * COV),
            "vW": vW,
            "pdl": ed["pdl_w"],
            "xTb": np.ascontiguousarray(x[sl].T.astype(BF)),
            "gw1": g_w1.astype(BF), "gb1": g_b1[:, None],
            "gw2": g_w2.astype(BF), "gb2": g_b2[:, None],
        })
    res_b = run_bass_kernel_spmd(nc_b, in_b, core_ids=list(range(NCORES)),
                                 trace=trace)
    out = np.concatenate(
        [np.ascontiguousarray(np.asarray(r["outT"]).T) for r in res_b.results],
        axis=0)
    return out, (res_a, res_b)


DEFAULT_CFG = Cfg(n=50000, e=500000, din=128)


def kernel(**inputs):
    out, _ = run(DEFAULT_CFG, inputs)
    return out.astype(np.float32)


# revision 21
# speedup vs baseline: 7.4046x; 1.0978x over previous
"""PointGNNConv (sum aggregation) on 8 Trainium2 NeuronCores.

Algebraic decomposition: with f_w = [f_w3; f_wx] (3+128 rows),
    msg_e = relu(edge_feat @ f_w + f_b) = relu(u[src_e] + v[dst_e])
    u_j = pos_j @ f_w3 + x_j @ f_wx
    v_i = (delta_i - pos_i) @ f_w3 + f_b

Sharding: dst-range sharding -- core c owns dst in [c*NPC, (c+1)*NPC).

Two NEFFs. Phase A computes per-node u/v (bf16) on each core's node slice.
Between NEFFs the host expands the tables into per-edge streams (pure row
gather / reordering, no FP) so phase B needs NO on-device gather (the SWDGE
per-edge gather of the original design serialized ~1.2ms of descriptor
generation on GpSimd).

Phase B edge layout (per core, edges sorted by dst, sections of 128 dsts):
 - DENSE: the first T edges of each dst go to column-aligned chunks -- slot
   p of dense chunk r holds the r-th edge of dst (sec_base+p). The add of
   v[dst] uses the *unexpanded* per-section v tile broadcast across chunks
   (no v stream), and the segment-sum over chunks is a DVE pairwise tree
   followed by ONE identity matmul per section (psum transpose-accumulate).
 - OVERFLOW: edges beyond T per dst (25% here) go to packed chunks with a
   dst-local label (pdl); selection matrices S[slot,w] = (pdl[slot]==w) are
   built on GpSimd (is_equal) and matmul-accumulated into the same psum
   window.
Everything is bf16 (DVE 2x/4x fast modes need 2-byte packed operands; fp8
runs at base rate), psum f32, output f32. The g-MLP + residual tail is
fused per supergroup (4 sections / 512 dsts).
"""
import sys

sys.path.insert(0, "/opt/trn_rl_repo")

import numpy as np
import ml_dtypes

import concourse.bass as bass
import concourse.mybir as mybir
import concourse.tile as tile
from concourse import bacc
from concourse.bass_utils import run_bass_kernel_spmd

BF16 = mybir.dt.bfloat16
F32 = mybir.dt.float32
FP8 = mybir.dt.float8e4
AF = mybir.ActivationFunctionType
ALU = mybir.AluOpType

NCORES = 8
BF = ml_dtypes.bfloat16


class Cfg:
    def __init__(self, n, e, din, dt=512, t_dense=7):
        self.N = n
        self.E = e
        self.DIN = din
        self.NPC = n // NCORES          # nodes (dsts) per core
        self.SEC = 128                  # dsts per section
        self.NSEC = -(-self.NPC // self.SEC)
        self.SG_SECS = 4                # sections per supergroup (psum window)
        self.NSG = -(-self.NSEC // self.SG_SECS)
        self.DT = dt                    # free-dim tile for phase A
        self.T = t_dense                # dense chunks (edges per dst) per sec
        self.COV = None                 # overflow chunks per section (data)


def _dtiles(total, dt):
    return [(i, min(dt, total - i)) for i in range(0, total, dt)]


# ---------------------------------------------------------------- phase A
def build_phase_a(cfg):
    nc = bacc.Bacc(num_devices=NCORES)
    D = cfg.DIN
    xT = nc.dram_tensor("xT", [D, cfg.NPC], BF16, kind="ExternalInput")
    posT = nc.dram_tensor("posT", [3, cfg.NPC], BF16, kind="ExternalInput")
    h_w1 = nc.dram_tensor("h_w1", [D, D], BF16, kind="ExternalInput")
    h_b1 = nc.dram_tensor("h_b1", [D, 1], F32, kind="ExternalInput")
    h_w2 = nc.dram_tensor("h_w2", [D, 3], BF16, kind="ExternalInput")
    h_b2 = nc.dram_tensor("h_b2", [3, 1], F32, kind="ExternalInput")
    f_w3 = nc.dram_tensor("f_w3", [3, D], BF16, kind="ExternalInput")
    f_wx = nc.dram_tensor("f_wx", [D, D], BF16, kind="ExternalInput")
    f_b = nc.dram_tensor("f_b", [D, 1], F32, kind="ExternalInput")
    uT = nc.dram_tensor("uT", [D, cfg.NPC], BF16, kind="ExternalOutput")
    vT = nc.dram_tensor("vT", [D, cfg.NPC], BF16, kind="ExternalOutput")

    with tile.TileContext(nc) as tc:
        with (
            tc.tile_pool(name="consts", bufs=1) as cp,
            tc.tile_pool(name="work", bufs=2) as wp,
            tc.tile_pool(name="psum", bufs=2, space="PSUM") as pp,
        ):
            w1_sb = cp.tile([D, D], BF16)
            nc.sync.dma_start(out=w1_sb[:], in_=h_w1[:])
            w2_sb = cp.tile([D, 3], BF16)
            nc.sync.dma_start(out=w2_sb[:], in_=h_w2[:])
            fw3_sb = cp.tile([3, D], BF16)
            nc.sync.dma_start(out=fw3_sb[:], in_=f_w3[:])
            fwx_sb = cp.tile([D, D], BF16)
            nc.sync.dma_start(out=fwx_sb[:], in_=f_wx[:])
            b1_sb = cp.tile([D, 1], F32)
            nc.sync.dma_start(out=b1_sb[:], in_=h_b1[:])
            b2_sb = cp.tile([3, 1], F32)
            nc.sync.dma_start(out=b2_sb[:], in_=h_b2[:])
            fb_sb = cp.tile([D, 1], F32)
            nc.sync.dma_start(out=fb_sb[:], in_=f_b[:])

            for off, w in _dtiles(cfg.NPC, cfg.DT):
                sl = slice(off, off + w)
                xt_t = wp.tile([D, cfg.DT], BF16, tag="xt_t")
                nc.gpsimd.dma_start(out=xt_t[:, :w], in_=xT[:, sl])
                pt_t = wp.tile([3, cfg.DT], BF16, tag="pt_t")
                nc.gpsimd.dma_start(out=pt_t[:, :w], in_=posT[:, sl])
                xt = xt_t[:, :w]
                pt = pt_t[:, :w]
                # t1 = relu(x @ h_w1 + h_b1)   [D, w]
                ps1 = pp.tile([D, cfg.DT], F32, tag="ps1")
                nc.tensor.matmul(out=ps1[:, :w], lhsT=w1_sb[:], rhs=xt,
                                 start=True, stop=True)
                t1 = wp.tile([D, cfg.DT], BF16, tag="t1")
                nc.scalar.activation(out=t1[:, :w], in_=ps1[:, :w], func=AF.Relu,
                                     bias=b1_sb[:])
                # delta = tanh(t1 @ h_w2 + h_b2)  [3, w]
                ps2 = pp.tile([3, cfg.DT], F32, tag="ps2")
                nc.tensor.matmul(out=ps2[:, :w], lhsT=w2_sb[:], rhs=t1[:, :w],
                                 start=True, stop=True)
                dmp = wp.tile([3, cfg.DT], BF16, tag="dmp")
                nc.scalar.activation(out=dmp[:, :w], in_=ps2[:, :w], func=AF.Tanh,
                                     bias=b2_sb[:])
                # diff = delta - pos  [3, w]
                nc.vector.tensor_tensor(out=dmp[:, :w], in0=dmp[:, :w],
                                        in1=pt, op=ALU.subtract)
                # u = pos @ f_w3 + x @ f_wx   [D, w]
                psu = pp.tile([D, cfg.DT], F32, tag="psu")
                nc.tensor.matmul(out=psu[:, :w], lhsT=fw3_sb[:], rhs=pt,
                                 start=True, stop=False)
                nc.tensor.matmul(out=psu[:, :w], lhsT=fwx_sb[:], rhs=xt,
                                 start=False, stop=True)
                ut = wp.tile([D, cfg.DT], BF16, tag="ut")
                nc.vector.tensor_copy(out=ut[:, :w], in_=psu[:, :w])
                nc.sync.dma_start(out=uT[:, sl], in_=ut[:, :w])
                # v = (delta - pos) @ f_w3 + f_b  [D, w]
                psv = pp.tile([D, cfg.DT], F32, tag="psv")
                nc.tensor.matmul(out=psv[:, :w], lhsT=fw3_sb[:], rhs=dmp[:, :w],
                                 start=True, stop=True)
                vt = wp.tile([D, cfg.DT], BF16, tag="vt")
                nc.vector.tensor_scalar_add(out=vt[:, :w], in0=psv[:, :w],
                                            scalar1=fb_sb[:])
                nc.sync.dma_start(out=vT[:, sl], in_=vt[:, :w])
    nc.finalize()
    return nc


def _fold_pairs(nc, wp, msg4, secs, T, D, tag):
    """Pairwise-sum msg4 [128, SGS, T, D] over the T axis -> [128, SGS, D].

    Returns an AP of shape [128, secs, D]. Emits ceil-tree tensor_tensor
    adds (bf16, packed last dim -> DVE fast mode)."""
    cur = msg4          # AP provider: current level tile, logical width wcur
    wcur = T
    lvl = 0
    while wcur > 1:
        half = wcur // 2
        nxt_w = half + (wcur % 2)
        nxt = wp.tile([128, msg4.shape[1], nxt_w, D], BF16,
                      tag=f"{tag}_l{lvl}")
        nc.vector.tensor_tensor(
            out=nxt[:, :secs, :half, :],
            in0=cur[:, :secs, 0:2 * half:2, :],
            in1=cur[:, :secs, 1:2 * half:2, :],
            op=ALU.add)
        if wcur % 2:
            # carry the odd tail chunk down a level
            nc.vector.tensor_copy(out=nxt[:, :secs, half:half + 1, :],
                                  in_=cur[:, :secs, wcur - 1:wcur, :])
        cur = nxt
        wcur = nxt_w
        lvl += 1
    return cur


# ---------------------------------------------------------------- phase B
def build_phase_b(cfg):
    nc = bacc.Bacc(num_devices=NCORES)
    D = cfg.DIN
    T = cfg.T
    COV = cfg.COV
    NSEC = cfg.NSEC
    SGS = cfg.SG_SECS

    u_d = nc.dram_tensor("u_d", [128, NSEC * T, D], BF16, kind="ExternalInput")
    u_o = nc.dram_tensor("u_o", [128, NSEC * COV, D], BF16, kind="ExternalInput")
    v_o = nc.dram_tensor("v_o", [128, NSEC * COV, D], BF16, kind="ExternalInput")
    vW = nc.dram_tensor("vW", [128, NSEC, D], BF16, kind="ExternalInput")
    s_o = nc.dram_tensor("s_o", [128, NSEC * COV, 128], FP8,
                         kind="ExternalInput")
    xTb = nc.dram_tensor("xTb", [D, cfg.NPC], BF16, kind="ExternalInput")
    gw1 = nc.dram_tensor("gw1", [D, D], BF16, kind="ExternalInput")
    gb1 = nc.dram_tensor("gb1", [D, 1], F32, kind="ExternalInput")
    gw2 = nc.dram_tensor("gw2", [D, D], BF16, kind="ExternalInput")
    gb2 = nc.dram_tensor("gb2", [D, 1], F32, kind="ExternalInput")
    outT = nc.dram_tensor("outT", [D, cfg.NPC], BF16, kind="ExternalOutput")

    ident = nc.inline_tensor(np.eye(128, dtype=BF), name="ident")

    with tile.TileContext(nc) as tc:
        with (
            tc.tile_pool(name="consts", bufs=1) as cp,
            tc.tile_pool(name="stream", bufs=2) as gp,
            tc.tile_pool(name="cwork", bufs=2) as wp,
            tc.tile_pool(name="psagg", bufs=2, space="PSUM") as pa,
            tc.tile_pool(name="psmlp", bufs=2, space="PSUM") as pm,
        ):
            ident_sb = cp.tile([128, 128], BF16)
            nc.sync.dma_start(out=ident_sb[:], in_=ident[:])
            gw1_sb = cp.tile([D, D], BF16)
            nc.sync.dma_start(out=gw1_sb[:], in_=gw1[:])
            gw2_sb = cp.tile([D, D], BF16)
            nc.sync.dma_start(out=gw2_sb[:], in_=gw2[:])
            gb1_sb = cp.tile([D, 1], F32)
            nc.sync.dma_start(out=gb1_sb[:], in_=gb1[:])
            gb2_sb = cp.tile([D, 1], F32)
            nc.sync.dma_start(out=gb2_sb[:], in_=gb2[:])

            for sg in range(cfg.NSG):
                s0 = sg * SGS
                s1 = min(s0 + SGS, NSEC)
                secs = s1 - s0

                ue_d = gp.tile([128, SGS, T, D], BF16, tag="ue_d")
                nc.gpsimd.dma_start(
                    out=ue_d[:, :secs, :, :].rearrange("p s r f -> p (s r) f"),
                    in_=u_d[:, s0 * T:s1 * T, :])
                v_sg = gp.tile([128, SGS, D], BF16, tag="v_sg")
                nc.gpsimd.dma_start(out=v_sg[:, :secs, :], in_=vW[:, s0:s1, :])
                ue_o = gp.tile([128, SGS * COV, D], BF16, tag="ue_o")
                nc.scalar.dma_start(out=ue_o[:, :secs * COV, :],
                                    in_=u_o[:, s0 * COV:s1 * COV, :])
                ve_o = gp.tile([128, SGS * COV, D], BF16, tag="ve_o")
                nc.scalar.dma_start(out=ve_o[:, :secs * COV, :],
                                    in_=v_o[:, s0 * COV:s1 * COV, :])
                st = gp.tile([128, SGS * COV, 128], FP8, tag="st")
                nc.scalar.dma_start(out=st[:, :secs * COV, :],
                                    in_=s_o[:, s0 * COV:s1 * COV, :])

                # dense: msg = relu(u + v_sec)   [p, s, r, f]
                msg_d = wp.tile([128, SGS, T, D], BF16, tag="msg_d")
                nc.vector.tensor_tensor(
                    out=msg_d[:, :secs, :, :],
                    in0=ue_d[:, :secs, :, :],
                    in1=v_sg[:, :secs, None, :].to_broadcast([128, secs, T, D]),
                    op=ALU.add)
                mdf = msg_d[:, :secs, :, :].rearrange("p s r f -> p (s r f)")
                nc.vector.tensor_relu(mdf, mdf)
                # one pairwise-fold level on DVE; the T//2 (+carry) partial
                # sums then go through identity matmuls (psum accumulate)
                npair = T // 2
                r1 = None
                if npair:
                    r1 = wp.tile([128, SGS, npair, D], BF16, tag="r1")
                    nc.vector.tensor_tensor(
                        out=r1[:, :secs, :, :],
                        in0=msg_d[:, :secs, 0:2 * npair:2, :],
                        in1=msg_d[:, :secs, 1:2 * npair:2, :],
                        op=ALU.add)

                # overflow: msg = relu(u + v)
                msg_o = wp.tile([128, SGS * COV, D], BF16, tag="msg_o")
                mof = msg_o[:, :secs * COV, :].rearrange("p c f -> p (c f)")
                nc.vector.tensor_tensor(
                    out=mof,
                    in0=ue_o[:, :secs * COV, :].rearrange("p c f -> p (c f)"),
                    in1=ve_o[:, :secs * COV, :].rearrange("p c f -> p (c f)"),
                    op=ALU.add)
                nc.vector.tensor_relu(mof, mof)

                # segment-sum into psum [feat, w]
                ps = pa.tile([D, SGS * cfg.SEC], F32, tag="psagg")
                for j in range(secs):
                    osl = slice(j * cfg.SEC, (j + 1) * cfg.SEC)
                    idchunks = [r1[:, j, t, :] for t in range(npair)]
                    if T % 2:
                        idchunks.append(msg_d[:, j, T - 1, :])
                    for t, ch in enumerate(idchunks):
                        nc.tensor.matmul(out=ps[:, osl], lhsT=ch,
                                         rhs=ident_sb[:], start=(t == 0),
                                         stop=(COV == 0 and
                                               t == len(idchunks) - 1))
                    for t in range(COV):
                        nc.tensor.matmul(
                            out=ps[:, osl],
                            lhsT=msg_o[:, j * COV + t, :],
                            rhs=st[:, j * COV + t, :],
                            start=False, stop=(t == COV - 1))
                aggt = wp.tile([D, SGS * cfg.SEC], BF16, tag="aggt")
                nc.scalar.activation(out=aggt[:, :secs * cfg.SEC],
                                     in_=ps[:, :secs * cfg.SEC], func=AF.Copy)

                # fused tail: out = x + relu(relu(agg@g_w1+g_b1)@g_w2+g_b2)
                n0 = s0 * cfg.SEC
                w = min(cfg.NPC, s1 * cfg.SEC) - n0
                nsl = slice(n0, n0 + w)
                ph1 = pm.tile([D, SGS * cfg.SEC], F32, tag="ph1")
                nc.tensor.matmul(out=ph1[:, :w], lhsT=gw1_sb[:],
                                 rhs=aggt[:, :w], start=True, stop=True)
                h1 = wp.tile([D, SGS * cfg.SEC], BF16, tag="h1")
                nc.scalar.activation(out=h1[:, :w], in_=ph1[:, :w], func=AF.Relu,
                                     bias=gb1_sb[:])
                ph2 = pm.tile([D, SGS * cfg.SEC], F32, tag="ph2")
                nc.tensor.matmul(out=ph2[:, :w], lhsT=gw2_sb[:],
                                 rhs=h1[:, :w], start=True, stop=True)
                h2 = wp.tile([D, SGS * cfg.SEC], F32, tag="h2")
                nc.scalar.activation(out=h2[:, :w], in_=ph2[:, :w], func=AF.Relu,
                                     bias=gb2_sb[:])
                xt = wp.tile([D, SGS * cfg.SEC], BF16, tag="xt")
                nc.sync.dma_start(out=xt[:, :w], in_=xTb[:, nsl])
                ob = wp.tile([D, SGS * cfg.SEC], BF16, tag="ob")
                nc.vector.tensor_tensor(out=ob[:, :w], in0=h2[:, :w],
                                        in1=xt[:, :w], op=ALU.add)
                nc.sync.dma_start(out=outT[:, nsl], in_=ob[:, :w])
    nc.finalize()
    return nc


# ------------------------------------------------------------ host side
def _preprocess(cfg, edge_index):
    """Sort edges by dst per core; dense/overflow slot assignment.

    Sets cfg.COV. Returns per-core dict with:
      idx_dense [NSEC*T*128] int64  (src node id per dense slot, -1 pad)
      idx_osrc  [NSEC*COV*128] int64 (src per overflow slot, -1 pad)
      idx_odst  [NSEC*COV*128] int64 (core-local dst per ov slot, -1 pad)
      pdl_w [128, NSEC*COV] bf16 (dst%128 per ov slot, -1 pad)
    """
    src = np.asarray(edge_index[0], dtype=np.int64)
    dst = np.asarray(edge_index[1], dtype=np.int64)
    order = np.argsort(dst, kind="stable")
    src, dst = src[order], dst[order]
    core = dst // cfg.NPC
    bounds = np.searchsorted(core, np.arange(NCORES + 1))
    T = cfg.T

    percore = []
    cov_max = 1
    for c in range(NCORES):
        lo, hi = bounds[c], bounds[c + 1]
        s, d = src[lo:hi], dst[lo:hi] - c * cfg.NPC
        deg = np.bincount(d, minlength=cfg.NPC)
        first = np.zeros(cfg.NPC, np.int64)
        np.cumsum(deg[:-1], out=first[1:])
        rank = np.arange(len(d)) - first[d]
        sec = d >> 7
        exc = np.maximum(deg - T, 0)
        exc_pad = np.zeros(cfg.NSEC * cfg.SEC, np.int64)
        exc_pad[:cfg.NPC] = exc
        sec_exc = exc_pad.reshape(cfg.NSEC, cfg.SEC).sum(1)
        cov_max = max(cov_max, int(np.ceil(sec_exc.max() / 128)))
        percore.append((s, d, sec, rank))
    cfg.COV = cov_max
    COV = cov_max

    out = []
    for c in range(NCORES):
        s, d, sec, rank = percore[c]
        md = rank < T
        idx_dense = np.full(cfg.NSEC * T * 128, -1, np.int64)
        slot_d = (sec[md] * T + rank[md]) * 128 + (d[md] & 127)
        idx_dense[slot_d] = s[md]

        mo = ~md
        sec_o = sec[mo]
        ostart = np.zeros(cfg.NSEC, np.int64)
        cnt_o = np.bincount(sec_o, minlength=cfg.NSEC)
        np.cumsum(cnt_o[:-1], out=ostart[1:])
        q = np.arange(len(sec_o)) - ostart[sec_o]
        slot_o = (sec_o * COV + (q >> 7)) * 128 + (q & 127)
        idx_osrc = np.full(cfg.NSEC * COV * 128, -1, np.int64)
        idx_odst = np.full(cfg.NSEC * COV * 128, -1, np.int64)
        idx_osrc[slot_o] = s[mo]
        idx_odst[slot_o] = d[mo]
        # selection matrices, fp8 {0,1}: S[slot, w] = (dst_local%128 == w)
        s_flat = np.zeros((cfg.NSEC * COV * 128, 128), np.float32)
        s_flat[slot_o, d[mo] & 127] = 1.0
        s_w = np.ascontiguousarray(
            s_flat.reshape(cfg.NSEC * COV, 128, 128).transpose(1, 0, 2)
        ).astype(ml_dtypes.float8_e4m3)
        out.append({"idx_dense": idx_dense, "idx_osrc": idx_osrc,
                    "idx_odst": idx_odst, "s_w": s_w})
    return out


def _expand(tbl, idx, ncols):
    """Gather rows of tbl by idx (zero row for idx<0), wrap to [128,ncols,D]."""
    rows = np.zeros((len(idx), tbl.shape[1]), dtype=tbl.dtype)
    valid = idx >= 0
    rows[valid] = tbl[idx[valid]]
    return np.ascontiguousarray(
        rows.reshape(ncols, 128, -1).transpose(1, 0, 2))


def run(cfg, inputs, trace=False):
    """Full pipeline. inputs: dict as from setup_inputs (numpy)."""
    x = np.asarray(inputs["x"], np.float32)
    pos = np.asarray(inputs["pos"], np.float32)
    edata = _preprocess(cfg, np.asarray(inputs["edge_index"]))

    h_w1 = np.asarray(inputs["h_w1"], np.float32)
    h_b1 = np.asarray(inputs["h_b1"], np.float32)
    h_w2 = np.asarray(inputs["h_w2"], np.float32)
    h_b2 = np.asarray(inputs["h_b2"], np.float32)
    f_w = np.asarray(inputs["f_w"], np.float32)
    f_b = np.asarray(inputs["f_b"], np.float32)
    g_w1 = np.asarray(inputs["g_w1"], np.float32)
    g_b1 = np.asarray(inputs["g_b1"], np.float32)
    g_w2 = np.asarray(inputs["g_w2"], np.float32)
    g_b2 = np.asarray(inputs["g_b2"], np.float32)

    nc_a = build_phase_a(cfg)
    in_a = []
    for c in range(NCORES):
        sl = slice(c * cfg.NPC, (c + 1) * cfg.NPC)
        in_a.append({
            "xT": np.ascontiguousarray(x[sl].T.astype(BF)),
            "posT": np.ascontiguousarray(pos[sl].T.astype(BF)),
            "h_w1": h_w1.astype(BF), "h_b1": h_b1[:, None],
            "h_w2": h_w2.astype(BF), "h_b2": h_b2[:, None],
            "f_w3": f_w[:3].astype(BF), "f_wx": f_w[3:].astype(BF),
            "f_b": f_b[:, None],
        })
    res_a = run_bass_kernel_spmd(nc_a, in_a, core_ids=list(range(NCORES)),
                                 trace=trace)
    # u table node-major over ALL nodes; v tables per-core node-major
    u_nm = np.concatenate(
        [np.ascontiguousarray(np.asarray(r["uT"]).T) for r in res_a.results],
        axis=0)
    v_nms = [np.ascontiguousarray(np.asarray(r["vT"]).T) for r in res_a.results]

    nc_b = build_phase_b(cfg)
    T, COV = cfg.T, cfg.COV
    in_b = []
    for c in range(NCORES):
        sl = slice(c * cfg.NPC, (c + 1) * cfg.NPC)
        ed = edata[c]
        v_nm = v_nms[c]
        # vW [128, NSEC, D]: vW[p, s] = v[s*128+p] (zero-pad past NPC)
        vpad = np.zeros((cfg.NSEC * cfg.SEC, cfg.DIN), dtype=v_nm.dtype)
        vpad[:cfg.NPC] = v_nm
        vW = np.ascontiguousarray(
            vpad.reshape(cfg.NSEC, 128, cfg.DIN).transpose(1, 0, 2))
        in_b.append({
            "u_d": _expand(u_nm, ed["idx_dense"], cfg.NSEC * T),
            "u_o": _expand(u_nm, ed["idx_osrc"], cfg.NSEC * COV),
            "v_o": _expand(v_nm, ed["idx_odst"], cfg.NSEC * COV),
            "vW": vW,
            "s_o": ed["s_w"],
            "xTb": np.ascontiguousarray(x[sl].T.astype(BF)),
            "gw1": g_w1.astype(BF), "gb1": g_b1[:, None],
            "gw2": g_w2.astype(BF), "gb2": g_b2[:, None],
        })
    res_b = run_bass_kernel_spmd(nc_b, in_b, core_ids=list(range(NCORES)),
                                 trace=trace)
    out = np.concatenate(
        [np.ascontiguousarray(np.asarray(r["outT"]).T) for r in res_b.results],
        axis=0)
    return out, (res_a, res_b)


DEFAULT_CFG = Cfg(n=50000, e=500000, din=128)


def kernel(**inputs):
    out, _ = run(DEFAULT_CFG, inputs)
    return out.astype(np.float32)
